# revision 1
# baseline (speedup 1.0000x reference)
"""GAT network (3 GATConv + mean-pool + MLP) as a Bass/Tile SPMD kernel on 8 TRN2 cores.

Sharding: nodes (and edges, partitioned by destination) split into 8 contiguous
node shards. Each core aggregates messages for its shard; bf16 feature tables
(with packed attention logit columns) are AllGathered between layers.

Self-contained: only needs numpy + the container's /opt/trn_rl_repo toolchain.
"""
import sys
import os

sys.path.insert(0, '/opt/trn_rl_repo')

import numpy as np
import ml_dtypes

H = 8
IN_DIM = 16


class Cfg:
    def __init__(self, n_nodes=50000, n_edges=800000, n_graphs=256,
                 n_cores=8, R=56, T=8, G=56, epi_chunk=7):
        self.n_nodes = n_nodes
        self.n_edges = n_edges
        self.n_graphs = n_graphs
        self.n_cores = n_cores
        self.shard = -(-n_nodes // (n_cores * 128)) * 128  # 128-aligned shard
        self.n_pad = self.shard * n_cores
        self.half = self.n_pad // 2
        self.C = self.shard // 128          # node tiles per shard
        self.R = R                          # ranges per pass (uniform)
        self.T = T                          # tiles (of 128 edges) per range
        self.G = G                          # tiles per gather supertile
        self.epi_chunk = epi_chunk          # node-tile cols per epilogue chunk
        assert self.C % epi_chunk == 0
        assert (R * T) % G == 0
        # layer dims
        self.F = [64, 128, 128]             # H * f_out per layer
        self.fph = [8, 16, 16]
        self.elem = [128, 256, 256]         # bf16 row width of gather tables
        self.nrhs = [72, 136, 136]          # msg cols + p cols
        self.PW = [128, 192, 192]           # evac width written to accum
        self.ACCW = 192                     # accum row width (f32, 768B)
        self.n_grp = -(-n_graphs // 128)    # graph groups for pooling

    def key(self):
        return (self.n_pad, self.n_graphs, self.n_cores, self.R, self.T,
                self.G, self.epi_chunk)


def _wrap16(a, reps=8):
    """j -> [j%16, j//16], replicated to 128 partitions."""
    w = a.reshape(-1, 16).T
    return np.ascontiguousarray(np.tile(w, (reps, 1)))


def _wrap128(a):
    """j -> [j%128, j//128]."""
    return np.ascontiguousarray(a.reshape(-1, 128).T)


def preprocess(edge_index, batch, cfg):
    """Build per-core edge-slot arrays. Returns (shared dict, per-core list)."""
    n, npad, shard, half = cfg.n_nodes, cfg.n_pad, cfg.shard, cfg.half
    R, T = cfg.R, cfg.T
    slot_e = T * 128                      # edges per range
    nslots = 2 * R * slot_e               # edge slots per core

    ei = np.asarray(edge_index)
    loops = np.arange(n, dtype=np.int64)
    src = np.concatenate([ei[0], loops])
    dst = np.concatenate([ei[1], loops])
    core = dst // shard

    per_core = []
    max_ranges = 0
    for c in range(cfg.n_cores):
        m = core == c
        srcs = src[m]
        dsts = dst[m] - c * shard
        idx_all = np.zeros(nslots, np.int16)
        aldix_all = np.zeros(2 * R * 128, np.int16)
        drel_all = np.full(nslots, -1.0, np.float32)
        epi = np.zeros(2 * shard, np.int16)
        for pi in range(2):
            pm = (srcs < half) if pi == 0 else (srcs >= half)
            s_p = srcs[pm]
            d_p = dsts[pm]
            order = np.argsort(d_p, kind='stable')
            s_p = s_p[order]
            d_p = d_p[order]
            cnt = np.bincount(d_p, minlength=shard)
            cum = np.concatenate([[0], np.cumsum(cnt)])
            bases = []
            s0 = 0
            while s0 < shard:
                e_node = int(np.searchsorted(cum, cum[s0] + slot_e,
                                             side='right')) - 1
                e_node = min(e_node, s0 + 128)
                assert e_node > s0, f"node {s0} has >{slot_e} edges"
                bases.append(s0)
                s0 = e_node
            nr = len(bases)
            max_ranges = max(max_ranges, nr)
            assert nr <= R, f"need R>={nr}"
            ends = bases[1:] + [shard]
            for r in range(R):
                b0 = bases[r] if r < nr else shard - 1
                aldix_all[(pi * R + r) * 128:(pi * R + r + 1) * 128] = \
                    np.minimum(b0 + np.arange(128), shard - 1)
                if r >= nr:
                    continue
                e0, e1 = int(cum[bases[r]]), int(cum[ends[r]])
                k = e1 - e0
                o = (pi * R + r) * slot_e
                idx_all[o:o + k] = s_p[e0:e1] - pi * half
                drel_all[o:o + k] = d_p[e0:e1] - bases[r]
            # epilogue slot-row index per node
            basearr = np.asarray(bases)
            rix = np.searchsorted(basearr, np.arange(shard), side='right') - 1
            epi[pi * shard:(pi + 1) * shard] = (
                (pi * R + rix) * 128 + (np.arange(shard) - basearr[rix])
            ).astype(np.int16)
        per_core.append(dict(
            idx=_wrap16(idx_all),
            aldix=_wrap16(aldix_all),
            drel=_wrap128(drel_all).astype(ml_dtypes.bfloat16),
            epi=_wrap16(epi),
        ))

    # batch-derived constants
    b = np.asarray(batch)
    cnt_g = np.bincount(b, minlength=cfg.n_graphs).astype(np.float32)
    rcnt_node = np.zeros(npad, np.float32)
    rcnt_node[:n] = 1.0 / np.maximum(cnt_g, 1.0)[b]
    gid = np.full(npad, -1, np.int64)
    gid[:n] = b
    for c in range(cfg.n_cores):
        sl = slice(c * shard, (c + 1) * shard)
        g_loc = gid[sl]
        grels = []
        for grp in range(cfg.n_grp):
            gr = np.where((g_loc >= grp * 128) & (g_loc < (grp + 1) * 128),
                          g_loc - grp * 128, -1).astype(np.float32)
            grels.append(_wrap128(gr).astype(ml_dtypes.bfloat16))
        per_core[c]['grel'] = grels
        per_core[c]['rcnt'] = _wrap128(rcnt_node[sl])
    return per_core, max_ranges


def make_weights(W1, a_src1, a_dst1, b1, W2, a_src2, a_dst2, b2,
                 W3, a_src3, a_dst3, b3, fc1_w, fc1_b, fc2_w, fc2_b, cfg):
    """Host-side weight packing: W' = [W | W@As | W@Ad] per layer."""
    def pack(W, a_s, a_d):
        f = a_s.shape[1]
        As = np.zeros((H * f, H), np.float32)
        Ad = np.zeros((H * f, H), np.float32)
        for h in range(H):
            As[h * f:(h + 1) * f, h] = a_s[h]
            Ad[h * f:(h + 1) * f, h] = a_d[h]
        out = np.concatenate([W, W @ As, W @ Ad], axis=1)
        width = 128 if out.shape[1] <= 128 else 256
        pad = np.zeros((out.shape[0], width - out.shape[1]), np.float32)
        return np.concatenate([out, pad], axis=1)

    bf = ml_dtypes.bfloat16
    sh = dict(
        w1p=pack(np.asarray(W1, np.float32), np.asarray(a_src1), np.asarray(a_dst1)).astype(bf),
        w2p=pack(np.asarray(W2, np.float32), np.asarray(a_src2), np.asarray(a_dst2)).astype(bf),
        w3p=pack(np.asarray(W3, np.float32), np.asarray(a_src3), np.asarray(a_dst3)).astype(bf),
        b1m=np.tile(np.asarray(b1, np.float32)[None, :], (128, 1)),
        b2m=np.tile(np.asarray(b2, np.float32)[None, :], (128, 1)),
        b3m=np.tile(np.asarray(b3, np.float32)[None, :], (128, 1)),
        iota=np.tile(np.arange(128, dtype=np.float32)[None, :], (128, 1)).astype(bf),
        ident=np.eye(128, dtype=np.float32).astype(bf),
        fc1=np.asarray(fc1_w, np.float32),
        fc1b=np.asarray(fc1_b, np.float32).reshape(32, 1),
        fc2=np.asarray(fc2_w, np.float32),
        fc2b=np.asarray(fc2_b, np.float32).reshape(1, 1),
    )
    return sh


def build_program(cfg):
    import concourse.bacc as bacc
    import concourse.bass as bass
    import concourse.tile as tile
    import concourse.mybir as mybir

    f32 = mybir.dt.float32
    bf16 = mybir.dt.bfloat16
    i16 = mybir.dt.int16
    AF = mybir.ActivationFunctionType
    OP = mybir.AluOpType

    npad, shard, C, half = cfg.n_pad, cfg.shard, cfg.C, cfg.half
    R, T, G = cfg.R, cfg.T, cfg.G
    n_sup = 2 * R * T // G          # gather supertiles per layer
    sup_pass = n_sup // 2
    rpg = G // T                    # ranges per supertile
    ECH = cfg.epi_chunk
    ACCW = cfg.ACCW

    nc = bacc.Bacc("TRN2", target_bir_lowering=False, debug=False,
                   num_devices=cfg.n_cores)

    def din(name, shape, dt):
        return nc.dram_tensor(name, shape, dt, kind="ExternalInput").ap()

    t_xT = din("xT", [IN_DIM, npad], bf16)
    t_xTs = din("xTs", [IN_DIM, shard], bf16)
    t_w = [din("w1p", [IN_DIM, 128], bf16),
           din("w2p", [64, 256], bf16),
           din("w3p", [128, 256], bf16)]
    t_b = [din("b1m", [128, 64], f32),
           din("b2m", [128, 128], f32),
           din("b3m", [128, 128], f32)]
    t_iota = din("iota", [128, 128], bf16)
    t_ident = din("ident", [128, 128], bf16)
    t_idx = din("idx", [128, 2 * R * T * 8], i16)
    t_aldix = din("aldix", [128, 2 * R * 8], i16)
    t_drel = din("drel", [128, 2 * R * T], bf16)
    t_epi = din("epi", [128, 2 * shard // 16], i16)
    t_grel = [din(f"grel{g}", [128, C], bf16) for g in range(cfg.n_grp)]
    t_rcnt = din("rcnt", [128, C], f32)
    t_fc1 = din("fc1", [128, 32], f32)
    t_fc1b = din("fc1b", [32, 1], f32)
    t_fc2 = din("fc2", [32, 1], f32)
    t_fc2b = din("fc2b", [1, 1], f32)
    t_out = nc.dram_tensor("out", [1, cfg.n_graphs], f32,
                           kind="ExternalOutput").ap()

    table1 = [nc.dram_tensor("table1lo", [half, 128], bf16),
              nc.dram_tensor("table1hi", [half, 128], bf16)]
    tables = [None,
              nc.dram_tensor("table2", [npad, 256], bf16, addr_space="Shared"),
              nc.dram_tensor("table3", [npad, 256], bf16, addr_space="Shared")]
    aldtabs = [nc.dram_tensor(f"aldtab{i}", [shard, 128], bf16)
               for i in range(3)]
    accum = nc.dram_tensor("accum", [2 * R * 128, ACCW], f32)

    with tile.TileContext(nc) as tc:
        with tc.tile_pool(name="cst", bufs=1) as cst, \
             tc.tile_pool(name="wk", bufs=1) as wk, \
             tc.tile_pool(name="ps", bufs=1, space="PSUM") as ps, \
             tc.tile_pool(name="dram", bufs=1, space="DRAM") as dram:

            # ---- persistent constants -------------------------------------
            iota_sb = cst.tile([128, 128], bf16)
            nc.sync.dma_start(iota_sb[:], t_iota)
            iota3 = iota_sb[:].rearrange("p (o j) -> p o j", o=1)
            ident_sb = cst.tile([128, 128], bf16)
            nc.sync.dma_start(ident_sb[:], t_ident)
            w_sb = []
            for l in range(3):
                w = cst.tile([t_w[l].shape[0], t_w[l].shape[1]], bf16,
                             name=f"w{l}_sb")
                nc.sync.dma_start(w[:], t_w[l])
                w_sb.append(w)
            b_sb = []
            for l in range(3):
                bt = cst.tile([128, t_b[l].shape[1]], f32, name=f"b{l}_sb")
                nc.sync.dma_start(bt[:], t_b[l])
                b_sb.append(bt)

            grel_sb = []
            for g in range(cfg.n_grp):
                gt_ = cst.tile([128, C], bf16, name=f"grel{g}_sb")
                nc.sync.dma_start(gt_[:], t_grel[g])
                grel_sb.append(gt_)
            rcnt_sb = cst.tile([128, C], f32)
            nc.sync.dma_start(rcnt_sb[:], t_rcnt)
            fc1_sb = cst.tile([128, 32], f32)
            nc.sync.dma_start(fc1_sb[:], t_fc1)
            fc1b_sb = cst.tile([32, 1], f32)
            nc.sync.dma_start(fc1b_sb[:], t_fc1b)
            fc2_sb = cst.tile([32, 1], f32)
            nc.sync.dma_start(fc2_sb[:], t_fc2)
            fc2b_sb = cst.tile([1, 1], f32)
            nc.sync.dma_start(fc2b_sb[:], t_fc2b)

            # ---- L1: every core computes the FULL table1 + its aldtab -----
            xTs_sb = wk.tile([128, shard], bf16, tag="xt", bufs=1)
            nc.sync.dma_start(xTs_sb[0:IN_DIM, :], t_xTs)
            NCHUNK = 512
            for nb in range(npad // NCHUNK):
                xc = wk.tile([IN_DIM, NCHUNK], bf16, tag="xc", bufs=2)
                nc.sync.dma_start(xc[:], t_xT[:, nb * NCHUNK:(nb + 1) * NCHUNK])
                for q in range(NCHUNK // 128):
                    pm = ps.tile([128, 128], f32, tag="pmisc", bufs=2)
                    nc.tensor.matmul(pm[:], lhsT=xc[:, q * 128:(q + 1) * 128],
                                     rhs=w_sb[0][:], start=True, stop=True)
                    hb = wk.tile([128, 128], bf16, tag="tb", bufs=3)
                    nc.vector.tensor_copy(hb[:], pm[:])
                    n0 = nb * NCHUNK + q * 128
                    t1 = table1[0] if n0 < half else table1[1]
                    eng = (nc.sync, nc.scalar)[q % 2]
                    eng.dma_start(t1.ap()[n0 % half:n0 % half + 128, :],
                                  hb[:])
            alds = wk.tile([128, C, 128], bf16, tag="alds", bufs=1)
            nc.vector.memset(alds[:], 0.0)
            for c in range(C):
                pa = ps.tile([128, 8], f32, tag="pmisc", bufs=2)
                nc.tensor.matmul(pa[:], lhsT=xTs_sb[0:IN_DIM, c * 128:(c + 1) * 128],
                                 rhs=w_sb[0][:, 72:80], start=True, stop=True)
                nc.vector.tensor_copy(alds[:, c, 0:8], pa[:])
            nc.sync.dma_start(
                aldtabs[0].ap().rearrange("(c p) j -> p c j", p=128), alds[:])

            pool_sb = cst.tile([128, cfg.n_grp * 128], f32)

            # ---- layers ---------------------------------------------------
            for l in range(3):
                F, fph, elem, nrhs = cfg.F[l], cfg.fph[l], cfg.elem[l], cfg.nrhs[l]
                PW = cfg.PW[l]
                tab = tables[l]

                # EDGE PHASE
                for sp in range(n_sup):
                    pi = 0 if sp < sup_pass else 1
                    if l == 0:
                        tab_ap = table1[pi].ap()
                    else:
                        tab_ap = tab.ap()[pi * half:(pi + 1) * half, :]
                    o8 = sp * G * 8
                    rpsup = G // T
                    idxs = wk.tile([128, G * 8], i16, tag="gidx", bufs=2)
                    nc.sync.dma_start(idxs[:], t_idx[:, o8:o8 + G * 8])
                    aix = wk.tile([128, rpsup * 8], i16, tag="aix", bufs=2)
                    nc.sync.dma_start(
                        aix[:], t_aldix[:, sp * rpsup * 8:(sp + 1) * rpsup * 8])
                    drl = wk.tile([128, G], bf16, tag="drel", bufs=2)
                    nc.sync.dma_start(drl[:], t_drel[:, sp * G:(sp + 1) * G])

                    gt = wk.tile([128, G, elem], bf16, tag="gt", bufs=2)
                    nc.gpsimd.dma_gather(gt[:], tab_ap, idxs[:],
                                         num_idxs=G * 128, num_idxs_reg=G * 128,
                                         elem_size=elem, single_packet=False)
                    aldr = wk.tile([128, rpsup, 128], bf16, tag="aldr", bufs=2)
                    nc.gpsimd.dma_gather(aldr[:], aldtabs[l].ap(), aix[:],
                                         num_idxs=rpsup * 128,
                                         num_idxs_reg=rpsup * 128,
                                         elem_size=128, single_packet=False)

                    # expand al_d per edge: per tile, psum_ald = S_T @ aldr
                    aldx = wk.tile([128, G, 8], f32, tag="aldx", bufs=2)
                    Ss = []
                    for rr in range(rpsup):
                        pald = ps.tile([128, T * 8], f32, tag="pmisc", bufs=2)
                        Se = wk.tile([128, T, 128], bf16, tag="S",
                                     bufs=rpsup)
                        d3 = drl[:, rr * T:(rr + 1) * T].rearrange(
                            "p (t o) -> p t o", o=1)
                        nc.vector.tensor_tensor(
                            out=Se[:], in0=d3.to_broadcast([128, T, 128]),
                            in1=iota3.to_broadcast([128, T, 128]),
                            op=OP.is_equal)
                        Ss.append(Se)
                        for t in range(T):
                            pst = ps.tile([128, 128], bf16, tag="stpool", bufs=2)
                            nc.tensor.transpose(pst[:], Se[:, t, :], ident_sb[:])
                            st_sb = wk.tile([128, 128], bf16, tag="st", bufs=3)
                            nc.scalar.activation(st_sb[:], pst[:], AF.Copy)
                            nc.tensor.matmul(pald[:, t * 8:(t + 1) * 8],
                                             lhsT=st_sb[:],
                                             rhs=aldr[:, rr, 0:8],
                                             start=True, stop=True)
                        nc.vector.tensor_copy(aldx[:, rr * T:(rr + 1) * T, :],
                                              pald[:])

                    # p = exp(lrelu(als+ald)) = max(exp(x), exp(0.2x))
                    lg = wk.tile([128, G, 8], f32, tag="lg", bufs=3)
                    nc.vector.tensor_tensor(out=lg[:], in0=gt[:, :, F:F + 8],
                                            in1=aldx[:], op=OP.add)
                    e1 = wk.tile([128, G, 8], f32, tag="lg", bufs=3)
                    nc.scalar.activation(e1[:], lg[:], AF.Exp)
                    e2 = wk.tile([128, G, 8], f32, tag="lg", bufs=3)
                    nc.scalar.activation(e2[:], lg[:], AF.Exp, scale=0.2)
                    nc.vector.tensor_tensor(out=gt[:, :, F:F + 8], in0=e1[:],
                                            in1=e2[:], op=OP.max)
                    # weight messages by p per head
                    for h in range(H):
                        nc.vector.tensor_tensor(
                            out=gt[:, :, h * fph:(h + 1) * fph],
                            in0=gt[:, :, h * fph:(h + 1) * fph],
                            in1=gt[:, :, F + h:F + h + 1].to_broadcast(
                                [128, G, fph]),
                            op=OP.mult)

                    ev = wk.tile([128, rpg, PW], f32, tag="ev", bufs=2)
                    for rr in range(rpg):
                        S = Ss[rr]
                        pacc = ps.tile([128, PW], f32, tag="pacc", bufs=3)
                        for t in range(T):
                            nc.tensor.matmul(pacc[:], lhsT=S[:, t, :],
                                             rhs=gt[:, rr * T + t, 0:PW],
                                             start=(t == 0), stop=(t == T - 1))
                        nc.vector.tensor_copy(ev[:, rr, :], pacc[:])
                    nc.sync.dma_start(
                        accum.ap()[sp * rpg * 128:(sp + 1) * rpg * 128, 0:PW]
                        .rearrange("(s p) w -> p s w", p=128),
                        ev[:])

                # EPILOGUE
                h_bfs = []
                for ch in range(C // ECH):
                    oc = ch * ECH * 8
                    e_lo = wk.tile([128, ECH * 8], i16, tag="ei", bufs=2)
                    nc.sync.dma_start(e_lo[:], t_epi[:, oc:oc + ECH * 8])
                    e_hi = wk.tile([128, ECH * 8], i16, tag="ei2", bufs=2)
                    nc.sync.dma_start(
                        e_hi[:],
                        t_epi[:, shard // 16 + oc:shard // 16 + oc + ECH * 8])
                    glo = wk.tile([128, ECH, PW], f32, tag="eg", bufs=2)
                    nc.gpsimd.dma_gather(glo[:], accum.ap()[:, 0:PW], e_lo[:],
                                         num_idxs=ECH * 128,
                                         num_idxs_reg=ECH * 128,
                                         elem_size=PW, elem_step=ACCW,
                                         single_packet=False)
                    ghi = wk.tile([128, ECH, PW], f32, tag="eg", bufs=2)
                    nc.gpsimd.dma_gather(ghi[:], accum.ap()[:, 0:PW], e_hi[:],
                                         num_idxs=ECH * 128,
                                         num_idxs_reg=ECH * 128,
                                         elem_size=PW, elem_step=ACCW,
                                         single_packet=False)
                    acc = wk.tile([128, ECH, nrhs], f32, tag="eacc", bufs=2)
                    nc.vector.tensor_tensor(out=acc[:], in0=glo[:, :, 0:nrhs],
                                            in1=ghi[:, :, 0:nrhs], op=OP.add)
                    rec = wk.tile([128, ECH, 8], f32, tag="rec", bufs=2)
                    nc.vector.tensor_scalar_add(rec[:], acc[:, :, F:F + 8],
                                                1e-30)
                    nc.vector.reciprocal(rec[:], rec[:])
                    for h in range(H):
                        nc.vector.tensor_tensor(
                            out=acc[:, :, h * fph:(h + 1) * fph],
                            in0=acc[:, :, h * fph:(h + 1) * fph],
                            in1=rec[:, :, h:h + 1].to_broadcast([128, ECH, fph]),
                            op=OP.mult)
                    nc.vector.tensor_tensor(
                        out=acc[:, :, 0:F], in0=acc[:, :, 0:F],
                        in1=b_sb[l][:].rearrange("p (o j) -> p o j", o=1)
                        .to_broadcast([128, ECH, F]),
                        op=OP.add)
                    # ELU: exp(min(x,0)) + max(x,0) - 1
                    t1 = wk.tile([128, ECH, F], f32, tag="et1", bufs=3)
                    nc.vector.tensor_scalar_min(t1[:], acc[:, :, 0:F], 0.0)
                    t2 = wk.tile([128, ECH, F], f32, tag="et1", bufs=3)
                    nc.scalar.activation(t2[:], t1[:], AF.Exp)
                    nc.vector.tensor_scalar_max(acc[:, :, 0:F],
                                                acc[:, :, 0:F], 0.0)
                    nc.vector.tensor_tensor(out=t2[:], in0=t2[:],
                                            in1=acc[:, :, 0:F], op=OP.add)
                    nc.vector.tensor_scalar_add(t2[:], t2[:], -1.0)
                    if l == 2:
                        nc.vector.tensor_tensor(
                            out=t2[:], in0=t2[:],
                            in1=rcnt_sb[:, ch * ECH:(ch + 1) * ECH]
                            .rearrange("p (t o) -> p t o", o=1)
                            .to_broadcast([128, ECH, F]),
                            op=OP.mult)
                    h_bf = wk.tile([128, ECH, F], bf16, tag="hbf",
                                   bufs=C // ECH)
                    nc.vector.tensor_copy(h_bf[:], t2[:])
                    h_bfs.append(h_bf)

                if l < 2:
                    # TABLE PHASE: transpose shard, matmul W', AllGather
                    xT_sb = wk.tile([128, shard], bf16, tag="xt", bufs=1)
                    for c in range(C):
                        pt = ps.tile([128, 128], bf16, tag="pmisc", bufs=2)
                        nc.tensor.transpose(pt[0:F, :],
                                            h_bfs[c // ECH][:, c % ECH, :],
                                            ident_sb[:])
                        nc.vector.tensor_copy(
                            xT_sb[0:F, c * 128:(c + 1) * 128], pt[0:F, :])
                    cin = dram.tile([shard, 256], bf16, tag="cin", bufs=1)
                    alds2 = wk.tile([128, C, 128], bf16, tag="alds", bufs=1)
                    nc.vector.memset(alds2[:], 0.0)
                    for c in range(C):
                        pm = ps.tile([128, 256], f32, tag="pmisc", bufs=2)
                        nc.tensor.matmul(pm[:],
                                         lhsT=xT_sb[0:F, c * 128:(c + 1) * 128],
                                         rhs=w_sb[l + 1][:],
                                         start=True, stop=True)
                        tb = wk.tile([128, 256], bf16, tag="tb", bufs=3)
                        nc.vector.tensor_copy(tb[:], pm[:])
                        nc.vector.tensor_copy(alds2[:, c, 0:8], pm[:, 136:144])
                        nc.sync.dma_start(cin[c * 128:(c + 1) * 128, :], tb[:])
                    nc.sync.dma_start(
                        aldtabs[l + 1].ap()
                        .rearrange("(c p) j -> p c j", p=128), alds2[:])
                    nc.gpsimd.collective_compute(
                        "AllGather", OP.bypass,
                        replica_groups=[list(range(cfg.n_cores))],
                        ins=[cin.opt()], outs=[tables[l + 1].ap()])
                else:
                    # POOLING
                    for grp in range(cfg.n_grp):
                        Sp = wk.tile([128, C, 128], bf16, tag="alds", bufs=1)
                        g3 = grel_sb[grp][:].rearrange("p (t o) -> p t o", o=1)
                        nc.vector.tensor_tensor(
                            out=Sp[:], in0=g3.to_broadcast([128, C, 128]),
                            in1=iota3.to_broadcast([128, C, 128]),
                            op=OP.is_equal)
                        pp = ps.tile([128, 128], f32, tag="stpool", bufs=2)
                        for c in range(C):
                            nc.tensor.matmul(pp[:],
                                             lhsT=h_bfs[c // ECH][:, c % ECH, :],
                                             rhs=Sp[:, c, :],
                                             start=(c == 0), stop=(c == C - 1))
                        nc.vector.tensor_copy(
                            pool_sb[:, grp * 128:(grp + 1) * 128], pp[:])

            # AllReduce pooled sums, then the MLP on every core
            cin2 = dram.tile([128, cfg.n_grp * 128], f32, tag="cin2", bufs=1)
            cred = dram.tile([128, cfg.n_grp * 128], f32, tag="cred", bufs=1)
            nc.sync.dma_start(cin2[:], pool_sb[:])
            nc.gpsimd.collective_compute(
                "AllReduce", OP.add,
                replica_groups=[list(range(cfg.n_cores))],
                ins=[cin2.opt()], outs=[cred.opt()])
            pool2 = wk.tile([128, cfg.n_grp * 128], f32, tag="pool2", bufs=1)
            nc.sync.dma_start(pool2[:], cred[:])
            pa = ps.tile([32, cfg.n_graphs], f32, tag="pmisc", bufs=2)
            nc.tensor.matmul(pa[:], lhsT=fc1_sb[:], rhs=pool2[:, 0:cfg.n_graphs],
                             start=True, stop=True)
            r1 = wk.tile([32, cfg.n_graphs], f32, tag="r1", bufs=1)
            nc.scalar.activation(r1[:], pa[:], AF.Relu, bias=fc1b_sb[:])
            pb = ps.tile([1, cfg.n_graphs], f32, tag="pmisc", bufs=2)
            nc.tensor.matmul(pb[:], lhsT=fc2_sb[:], rhs=r1[:],
                             start=True, stop=True)
            ob = wk.tile([1, cfg.n_graphs], f32, tag="ob", bufs=1)
            nc.scalar.activation(ob[:], pb[:], AF.Identity, bias=fc2b_sb[:])
            nc.sync.dma_start(t_out, ob[:])

    nc.compile()
    return nc


_PROG_CACHE = {}


def run_gat(x, edge_index, batch, weights, cfg, trace=False):
    """weights: dict from make_weights. Returns (out [n_graphs], exec_ns)."""
    from concourse.bass_utils import run_bass_kernel_spmd

    bf = ml_dtypes.bfloat16
    n = cfg.n_nodes
    x_pad = np.zeros((cfg.n_pad, IN_DIM), np.float32)
    x_pad[:n] = np.asarray(x, np.float32)
    xT = np.ascontiguousarray(x_pad.T).astype(bf)

    per_core, _ = preprocess(edge_index, batch, cfg)

    key = cfg.key()
    if key not in _PROG_CACHE:
        _PROG_CACHE[key] = build_program(cfg)
    nc = _PROG_CACHE[key]

    in_maps = []
    for c in range(cfg.n_cores):
        pc = per_core[c]
        m = dict(
            xT=xT,
            xTs=np.ascontiguousarray(xT[:, c * cfg.shard:(c + 1) * cfg.shard]),
            w1p=weights['w1p'], w2p=weights['w2p'], w3p=weights['w3p'],
            b1m=weights['b1m'], b2m=weights['b2m'], b3m=weights['b3m'],
            iota=weights['iota'], ident=weights['ident'],
            idx=pc['idx'], aldix=pc['aldix'], drel=pc['drel'], epi=pc['epi'],
            rcnt=pc['rcnt'],
            fc1=weights['fc1'], fc1b=weights['fc1b'],
            fc2=weights['fc2'], fc2b=weights['fc2b'],
        )
        for g in range(cfg.n_grp):
            m[f"grel{g}"] = pc['grel'][g]
        in_maps.append(m)

    res = run_bass_kernel_spmd(nc, in_maps, core_ids=list(range(cfg.n_cores)),
                               trace=trace)
    out = np.asarray(res.results[0]['out']).reshape(cfg.n_graphs, 1)
    run_gat.last_res = res
    return out, res.exec_time_ns


# ----------------------------------------------------------------------------
# Harness entrypoint: full (unsharded) inputs -> full output [N_GRAPHS, 1].
# Shards edges by destination across the 8 NeuronCores internally.
# ----------------------------------------------------------------------------
_DEF_CFG = None


def kernel(x, edge_index, batch,
           W1, a_src1, a_dst1, b1,
           W2, a_src2, a_dst2, b2,
           W3, a_src3, a_dst3, b3,
           fc1_w, fc1_b, fc2_w, fc2_b):
    global _DEF_CFG
    if _DEF_CFG is None:
        _DEF_CFG = Cfg()  # 50000 nodes / 800000 edges / 256 graphs / 8 cores
    cfg = _DEF_CFG
    weights = make_weights(W1, a_src1, a_dst1, b1, W2, a_src2, a_dst2, b2,
                           W3, a_src3, a_dst3, b3, fc1_w, fc1_b, fc2_w, fc2_b,
                           cfg)
    trace = bool(int(os.environ.get("GAT_BASS_TRACE", "0")))
    out, ns = run_gat(np.asarray(x), np.asarray(edge_index),
                      np.asarray(batch), weights, cfg, trace=trace)
    kernel.exec_time_ns = ns
    return out.astype(np.float32)



# revision 5
# speedup vs baseline: 1.3602x; 1.3602x over previous
"""GAT network (3 GATConv + mean-pool + MLP) as a Bass/Tile SPMD kernel on 8 TRN2 cores.

Sharding: nodes (and edges, partitioned by destination) split into 8 contiguous
node shards. Each core aggregates messages for its shard; bf16 feature tables
(with packed attention logit columns) are AllGathered between layers.

Self-contained: only needs numpy + the container's /opt/trn_rl_repo toolchain.
"""
import sys
import os

sys.path.insert(0, '/opt/trn_rl_repo')

import numpy as np
import ml_dtypes

H = 8
IN_DIM = 16


class Cfg:
    def __init__(self, n_nodes=50000, n_edges=800000, n_graphs=256,
                 n_cores=8, R=56, T=8, G=56, epi_chunk=7):
        self.n_nodes = n_nodes
        self.n_edges = n_edges
        self.n_graphs = n_graphs
        self.n_cores = n_cores
        self.shard = -(-n_nodes // (n_cores * 128)) * 128  # 128-aligned shard
        self.n_pad = self.shard * n_cores
        self.half = self.n_pad // 2
        self.C = self.shard // 128          # node tiles per shard
        self.R = R                          # ranges per pass (uniform)
        self.T = T                          # tiles (of 128 edges) per range
        self.G = G                          # tiles per gather supertile
        self.epi_chunk = epi_chunk          # node-tile cols per epilogue chunk
        assert self.C % epi_chunk == 0
        assert (R * T) % G == 0
        # layer dims
        self.F = [64, 128, 128]             # H * f_out per layer
        self.fph = [8, 16, 16]
        self.elem = [128, 256, 256]         # bf16 row width of gather tables
        self.nrhs = [72, 136, 136]          # msg cols + p cols
        self.PW = [128, 192, 192]           # evac width written to accum
        self.ACCW = 192                     # accum row width (f32, 768B)
        self.n_grp = -(-n_graphs // 128)    # graph groups for pooling

    def key(self):
        return (self.n_pad, self.n_graphs, self.n_cores, self.R, self.T,
                self.G, self.epi_chunk)


def _wrap16(a, reps=8):
    """j -> [j%16, j//16], replicated to 128 partitions."""
    w = a.reshape(-1, 16).T
    return np.ascontiguousarray(np.tile(w, (reps, 1)))


def _wrap128(a):
    """j -> [j%128, j//128]."""
    return np.ascontiguousarray(a.reshape(-1, 128).T)


def preprocess(edge_index, batch, cfg):
    """Build per-core edge-slot arrays. Returns (shared dict, per-core list)."""
    n, npad, shard, half = cfg.n_nodes, cfg.n_pad, cfg.shard, cfg.half
    R, T = cfg.R, cfg.T
    slot_e = T * 128                      # edges per range
    nslots = 2 * R * slot_e               # edge slots per core

    ei = np.asarray(edge_index)
    loops = np.arange(n, dtype=np.int64)
    src = np.concatenate([ei[0], loops])
    dst = np.concatenate([ei[1], loops])
    core = dst // shard

    per_core = []
    max_ranges = 0
    for c in range(cfg.n_cores):
        m = core == c
        srcs = src[m]
        dsts = dst[m] - c * shard
        idx_all = np.zeros(nslots, np.int16)
        aldix_all = np.zeros(2 * R * 128, np.int16)
        drel_all = np.full(nslots, -1.0, np.float32)
        epi = np.zeros(2 * shard, np.int16)
        for pi in range(2):
            pm = (srcs < half) if pi == 0 else (srcs >= half)
            s_p = srcs[pm]
            d_p = dsts[pm]
            order = np.argsort(d_p, kind='stable')
            s_p = s_p[order]
            d_p = d_p[order]
            cnt = np.bincount(d_p, minlength=shard)
            cum = np.concatenate([[0], np.cumsum(cnt)])
            bases = []
            s0 = 0
            while s0 < shard:
                e_node = int(np.searchsorted(cum, cum[s0] + slot_e,
                                             side='right')) - 1
                e_node = min(e_node, s0 + 128)
                assert e_node > s0, f"node {s0} has >{slot_e} edges"
                bases.append(s0)
                s0 = e_node
            nr = len(bases)
            max_ranges = max(max_ranges, nr)
            assert nr <= R, f"need R>={nr}"
            ends = bases[1:] + [shard]
            for r in range(R):
                b0 = bases[r] if r < nr else shard - 1
                aldix_all[(pi * R + r) * 128:(pi * R + r + 1) * 128] = \
                    np.minimum(b0 + np.arange(128), shard - 1)
                if r >= nr:
                    continue
                e0, e1 = int(cum[bases[r]]), int(cum[ends[r]])
                k = e1 - e0
                o = (pi * R + r) * slot_e
                idx_all[o:o + k] = s_p[e0:e1] - pi * half
                drel_all[o:o + k] = d_p[e0:e1] - bases[r]
            # epilogue slot-row index per node
            basearr = np.asarray(bases)
            rix = np.searchsorted(basearr, np.arange(shard), side='right') - 1
            epi[pi * shard:(pi + 1) * shard] = (
                (pi * R + rix) * 128 + (np.arange(shard) - basearr[rix])
            ).astype(np.int16)
        per_core.append(dict(
            idx=_wrap16(idx_all),
            aldix=_wrap16(aldix_all),
            drel=_wrap128(drel_all).astype(ml_dtypes.bfloat16),
            epi=_wrap16(epi),
        ))

    # batch-derived constants
    b = np.asarray(batch)
    cnt_g = np.bincount(b, minlength=cfg.n_graphs).astype(np.float32)
    rcnt_node = np.zeros(npad, np.float32)
    rcnt_node[:n] = 1.0 / np.maximum(cnt_g, 1.0)[b]
    gid = np.full(npad, -1, np.int64)
    gid[:n] = b
    for c in range(cfg.n_cores):
        sl = slice(c * shard, (c + 1) * shard)
        g_loc = gid[sl]
        grels = []
        for grp in range(cfg.n_grp):
            gr = np.where((g_loc >= grp * 128) & (g_loc < (grp + 1) * 128),
                          g_loc - grp * 128, -1).astype(np.float32)
            grels.append(_wrap128(gr).astype(ml_dtypes.bfloat16))
        per_core[c]['grel'] = grels
        per_core[c]['rcnt'] = _wrap128(rcnt_node[sl])
    return per_core, max_ranges


def make_weights(W1, a_src1, a_dst1, b1, W2, a_src2, a_dst2, b2,
                 W3, a_src3, a_dst3, b3, fc1_w, fc1_b, fc2_w, fc2_b, cfg):
    """Host-side weight packing: W' = [W | W@As | W@Ad] per layer."""
    def pack(W, a_s, a_d):
        f = a_s.shape[1]
        As = np.zeros((H * f, H), np.float32)
        Ad = np.zeros((H * f, H), np.float32)
        for h in range(H):
            As[h * f:(h + 1) * f, h] = a_s[h]
            Ad[h * f:(h + 1) * f, h] = a_d[h]
        out = np.concatenate([W, W @ As, W @ Ad], axis=1)
        width = 128 if out.shape[1] <= 128 else 256
        pad = np.zeros((out.shape[0], width - out.shape[1]), np.float32)
        return np.concatenate([out, pad], axis=1)

    bf = ml_dtypes.bfloat16
    sh = dict(
        w1p=pack(np.asarray(W1, np.float32), np.asarray(a_src1), np.asarray(a_dst1)).astype(bf),
        w2p=pack(np.asarray(W2, np.float32), np.asarray(a_src2), np.asarray(a_dst2)).astype(bf),
        w3p=pack(np.asarray(W3, np.float32), np.asarray(a_src3), np.asarray(a_dst3)).astype(bf),
        b1m=np.tile(np.asarray(b1, np.float32)[None, :], (128, 1)),
        b2m=np.tile(np.asarray(b2, np.float32)[None, :], (128, 1)),
        b3m=np.tile(np.asarray(b3, np.float32)[None, :], (128, 1)),
        iota=np.tile(np.arange(128, dtype=np.float32)[None, :], (128, 1)).astype(bf),
        ident=np.eye(128, dtype=np.float32).astype(bf),
        fc1=np.asarray(fc1_w, np.float32),
        fc1b=np.asarray(fc1_b, np.float32).reshape(32, 1),
        fc2=np.asarray(fc2_w, np.float32),
        fc2b=np.asarray(fc2_b, np.float32).reshape(1, 1),
    )
    return sh


def build_program(cfg):
    import concourse.bacc as bacc
    import concourse.bass as bass
    import concourse.tile as tile
    import concourse.mybir as mybir

    f32 = mybir.dt.float32
    bf16 = mybir.dt.bfloat16
    i16 = mybir.dt.int16
    AF = mybir.ActivationFunctionType
    OP = mybir.AluOpType

    npad, shard, C, half = cfg.n_pad, cfg.shard, cfg.C, cfg.half
    R, T, G = cfg.R, cfg.T, cfg.G
    n_sup = 2 * R * T // G          # gather supertiles per layer
    sup_pass = n_sup // 2
    rpg = G // T                    # ranges per supertile
    ECH = cfg.epi_chunk
    ACCW = cfg.ACCW

    nc = bacc.Bacc("TRN2", target_bir_lowering=False, debug=False,
                   num_devices=cfg.n_cores, num_swdge_queues=4)
    _qctr = [0]

    def next_q():
        q = _qctr[0] % 4
        _qctr[0] += 1
        return q

    def din(name, shape, dt):
        return nc.dram_tensor(name, shape, dt, kind="ExternalInput").ap()

    t_xT = din("xT", [IN_DIM, npad], bf16)
    t_xTs = din("xTs", [IN_DIM, shard], bf16)
    t_w = [din("w1p", [IN_DIM, 128], bf16),
           din("w2p", [64, 256], bf16),
           din("w3p", [128, 256], bf16)]
    t_b = [din("b1m", [128, 64], f32),
           din("b2m", [128, 128], f32),
           din("b3m", [128, 128], f32)]
    t_iota = din("iota", [128, 128], bf16)
    t_ident = din("ident", [128, 128], bf16)
    t_idx = din("idx", [128, 2 * R * T * 8], i16)
    t_aldix = din("aldix", [128, 2 * R * 8], i16)
    t_drel = din("drel", [128, 2 * R * T], bf16)
    t_epi = din("epi", [128, 2 * shard // 16], i16)
    t_grel = [din(f"grel{g}", [128, C], bf16) for g in range(cfg.n_grp)]
    t_rcnt = din("rcnt", [128, C], f32)
    t_fc1 = din("fc1", [128, 32], f32)
    t_fc1b = din("fc1b", [32, 1], f32)
    t_fc2 = din("fc2", [32, 1], f32)
    t_fc2b = din("fc2b", [1, 1], f32)
    t_out = nc.dram_tensor("out", [1, cfg.n_graphs], f32,
                           kind="ExternalOutput").ap()

    table1 = [nc.dram_tensor("table1lo", [half, 128], bf16),
              nc.dram_tensor("table1hi", [half, 128], bf16)]
    tables = [None,
              nc.dram_tensor("table2", [npad, 256], bf16, addr_space="Shared"),
              nc.dram_tensor("table3", [npad, 256], bf16, addr_space="Shared")]
    aldtabs = [nc.dram_tensor(f"aldtab{i}", [shard, 128], bf16)
               for i in range(3)]
    accum = nc.dram_tensor("accum", [2 * R * 128, ACCW], f32)

    with tile.TileContext(nc) as tc:
        with tc.tile_pool(name="cst", bufs=1) as cst, \
             tc.tile_pool(name="wk", bufs=1) as wk, \
             tc.tile_pool(name="ps", bufs=1, space="PSUM") as ps, \
             tc.tile_pool(name="dram", bufs=1, space="DRAM") as dram:

            # ---- persistent constants -------------------------------------
            iota_sb = cst.tile([128, 128], bf16)
            nc.sync.dma_start(iota_sb[:], t_iota)
            iota3 = iota_sb[:].rearrange("p (o j) -> p o j", o=1)
            ident_sb = cst.tile([128, 128], bf16)
            nc.sync.dma_start(ident_sb[:], t_ident)
            w_sb = []
            for l in range(3):
                w = cst.tile([t_w[l].shape[0], t_w[l].shape[1]], bf16,
                             name=f"w{l}_sb")
                nc.sync.dma_start(w[:], t_w[l])
                w_sb.append(w)
            b_sb = []
            for l in range(3):
                bt = cst.tile([128, t_b[l].shape[1]], f32, name=f"b{l}_sb")
                nc.sync.dma_start(bt[:], t_b[l])
                b_sb.append(bt)

            grel_sb = []
            for g in range(cfg.n_grp):
                gt_ = cst.tile([128, C], bf16, name=f"grel{g}_sb")
                nc.sync.dma_start(gt_[:], t_grel[g])
                grel_sb.append(gt_)
            rcnt_sb = cst.tile([128, C], f32)
            nc.sync.dma_start(rcnt_sb[:], t_rcnt)
            fc1_sb = cst.tile([128, 32], f32)
            nc.sync.dma_start(fc1_sb[:], t_fc1)
            fc1b_sb = cst.tile([32, 1], f32)
            nc.sync.dma_start(fc1b_sb[:], t_fc1b)
            fc2_sb = cst.tile([32, 1], f32)
            nc.sync.dma_start(fc2_sb[:], t_fc2)
            fc2b_sb = cst.tile([1, 1], f32)
            nc.sync.dma_start(fc2b_sb[:], t_fc2b)

            # ---- L1: every core computes the FULL table1 + its aldtab -----
            xTs_sb = wk.tile([128, shard], bf16, tag="xt", bufs=1)
            nc.sync.dma_start(xTs_sb[0:IN_DIM, :], t_xTs)
            NCHUNK = 512
            for nb in range(npad // NCHUNK):
                xc = wk.tile([IN_DIM, NCHUNK], bf16, tag="xc", bufs=2)
                nc.sync.dma_start(xc[:], t_xT[:, nb * NCHUNK:(nb + 1) * NCHUNK])
                for q in range(NCHUNK // 128):
                    pm = ps.tile([128, 128], f32, tag="pmisc", bufs=2)
                    nc.tensor.matmul(pm[:], lhsT=xc[:, q * 128:(q + 1) * 128],
                                     rhs=w_sb[0][:], start=True, stop=True)
                    hb = wk.tile([128, 128], bf16, tag="tb", bufs=3)
                    nc.vector.tensor_copy(hb[:], pm[:])
                    n0 = nb * NCHUNK + q * 128
                    t1 = table1[0] if n0 < half else table1[1]
                    eng = (nc.sync, nc.scalar)[q % 2]
                    eng.dma_start(t1.ap()[n0 % half:n0 % half + 128, :],
                                  hb[:])
            alds = wk.tile([128, C, 128], bf16, tag="alds", bufs=1)
            nc.vector.memset(alds[:], 0.0)
            for c in range(C):
                pa = ps.tile([128, 8], f32, tag="pmisc", bufs=2)
                nc.tensor.matmul(pa[:], lhsT=xTs_sb[0:IN_DIM, c * 128:(c + 1) * 128],
                                 rhs=w_sb[0][:, 72:80], start=True, stop=True)
                nc.vector.tensor_copy(alds[:, c, 0:8], pa[:])
            nc.sync.dma_start(
                aldtabs[0].ap().rearrange("(c p) j -> p c j", p=128), alds[:])

            pool_sb = cst.tile([128, cfg.n_grp * 128], f32)

            # ---- layers ---------------------------------------------------
            for l in range(3):
                F, fph, elem, nrhs = cfg.F[l], cfg.fph[l], cfg.elem[l], cfg.nrhs[l]
                PW = cfg.PW[l]
                tab = tables[l]

                # EDGE PHASE
                for sp in range(n_sup):
                    pi = 0 if sp < sup_pass else 1
                    if l == 0:
                        tab_ap = table1[pi].ap()
                    else:
                        tab_ap = tab.ap()[pi * half:(pi + 1) * half, :]
                    o8 = sp * G * 8
                    rpsup = G // T
                    idxs = wk.tile([128, G * 8], i16, tag="gidx", bufs=2)
                    nc.sync.dma_start(idxs[:], t_idx[:, o8:o8 + G * 8])
                    aix = wk.tile([128, rpsup * 8], i16, tag="aix", bufs=2)
                    nc.sync.dma_start(
                        aix[:], t_aldix[:, sp * rpsup * 8:(sp + 1) * rpsup * 8])
                    drl = wk.tile([128, G], bf16, tag="drel", bufs=2)
                    nc.sync.dma_start(drl[:], t_drel[:, sp * G:(sp + 1) * G])

                    gt = wk.tile([128, G, elem], bf16, tag="gt", bufs=2)
                    half_g = G // 2
                    nc.gpsimd.dma_gather(gt[:, 0:half_g, :], tab_ap,
                                         idxs[:, 0:half_g * 8],
                                         num_idxs=half_g * 128,
                                         num_idxs_reg=half_g * 128,
                                         elem_size=elem, single_packet=False,
                                         queue_num=next_q())
                    nc.gpsimd.dma_gather(gt[:, half_g:G, :], tab_ap,
                                         idxs[:, half_g * 8:G * 8],
                                         num_idxs=(G - half_g) * 128,
                                         num_idxs_reg=(G - half_g) * 128,
                                         elem_size=elem, single_packet=False,
                                         queue_num=next_q())
                    aldr = wk.tile([128, rpsup, 128], bf16, tag="aldr", bufs=2)
                    nc.gpsimd.dma_gather(aldr[:], aldtabs[l].ap(), aix[:],
                                         num_idxs=rpsup * 128,
                                         num_idxs_reg=rpsup * 128,
                                         elem_size=128, single_packet=False,
                                         queue_num=next_q())

                    # expand al_d per edge: per tile, psum_ald = S_T @ aldr
                    aldx = wk.tile([128, G, 8], f32, tag="aldx", bufs=2)
                    Ss = []
                    for rr in range(rpsup):
                        pald = ps.tile([128, T * 8], f32, tag="pmisc", bufs=2)
                        Se = wk.tile([128, T, 128], bf16, tag="S",
                                     bufs=rpsup)
                        d3 = drl[:, rr * T:(rr + 1) * T].rearrange(
                            "p (t o) -> p t o", o=1)
                        nc.vector.tensor_tensor(
                            out=Se[:], in0=d3.to_broadcast([128, T, 128]),
                            in1=iota3.to_broadcast([128, T, 128]),
                            op=OP.is_equal)
                        Ss.append(Se)
                        for t in range(T):
                            pst = ps.tile([128, 128], bf16, tag="stpool", bufs=2)
                            nc.tensor.transpose(pst[:], Se[:, t, :], ident_sb[:])
                            st_sb = wk.tile([128, 128], bf16, tag="st", bufs=3)
                            nc.scalar.activation(st_sb[:], pst[:], AF.Copy)
                            nc.tensor.matmul(pald[:, t * 8:(t + 1) * 8],
                                             lhsT=st_sb[:],
                                             rhs=aldr[:, rr, 0:8],
                                             start=True, stop=True)
                        nc.vector.tensor_copy(aldx[:, rr * T:(rr + 1) * T, :],
                                              pald[:])

                    # p = exp(lrelu(als+ald)) = max(exp(x), exp(0.2x))
                    lg = wk.tile([128, G, 8], f32, tag="lg", bufs=3)
                    nc.vector.tensor_tensor(out=lg[:], in0=gt[:, :, F:F + 8],
                                            in1=aldx[:], op=OP.add)
                    e1 = wk.tile([128, G, 8], f32, tag="lg", bufs=3)
                    nc.scalar.activation(e1[:], lg[:], AF.Exp)
                    e2 = wk.tile([128, G, 8], f32, tag="lg", bufs=3)
                    nc.scalar.activation(e2[:], lg[:], AF.Exp, scale=0.2)
                    nc.vector.tensor_tensor(out=gt[:, :, F:F + 8], in0=e1[:],
                                            in1=e2[:], op=OP.max)
                    # weight messages by p per head
                    for h in range(H):
                        nc.vector.tensor_tensor(
                            out=gt[:, :, h * fph:(h + 1) * fph],
                            in0=gt[:, :, h * fph:(h + 1) * fph],
                            in1=gt[:, :, F + h:F + h + 1].to_broadcast(
                                [128, G, fph]),
                            op=OP.mult)

                    ev = wk.tile([128, rpg, PW], f32, tag="ev", bufs=2)
                    for rr in range(rpg):
                        S = Ss[rr]
                        pacc = ps.tile([128, PW], f32, tag="pacc", bufs=3)
                        for t in range(T):
                            nc.tensor.matmul(pacc[:], lhsT=S[:, t, :],
                                             rhs=gt[:, rr * T + t, 0:PW],
                                             start=(t == 0), stop=(t == T - 1))
                        nc.vector.tensor_copy(ev[:, rr, :], pacc[:])
                    nc.sync.dma_start(
                        accum.ap()[sp * rpg * 128:(sp + 1) * rpg * 128, 0:PW]
                        .rearrange("(s p) w -> p s w", p=128),
                        ev[:])

                # EPILOGUE
                h_bfs = []
                for ch in range(C // ECH):
                    oc = ch * ECH * 8
                    e_lo = wk.tile([128, ECH * 8], i16, tag="ei", bufs=2)
                    nc.sync.dma_start(e_lo[:], t_epi[:, oc:oc + ECH * 8])
                    e_hi = wk.tile([128, ECH * 8], i16, tag="ei2", bufs=2)
                    nc.sync.dma_start(
                        e_hi[:],
                        t_epi[:, shard // 16 + oc:shard // 16 + oc + ECH * 8])
                    glo = wk.tile([128, ECH, PW], f32, tag="eg", bufs=2)
                    nc.gpsimd.dma_gather(glo[:], accum.ap()[:, 0:PW], e_lo[:],
                                         num_idxs=ECH * 128,
                                         num_idxs_reg=ECH * 128,
                                         elem_size=PW, elem_step=ACCW,
                                         single_packet=False,
                                         queue_num=next_q())
                    ghi = wk.tile([128, ECH, PW], f32, tag="eg", bufs=2)
                    nc.gpsimd.dma_gather(ghi[:], accum.ap()[:, 0:PW], e_hi[:],
                                         num_idxs=ECH * 128,
                                         num_idxs_reg=ECH * 128,
                                         elem_size=PW, elem_step=ACCW,
                                         single_packet=False,
                                         queue_num=next_q())
                    acc = wk.tile([128, ECH, nrhs], f32, tag="eacc", bufs=2)
                    nc.vector.tensor_tensor(out=acc[:], in0=glo[:, :, 0:nrhs],
                                            in1=ghi[:, :, 0:nrhs], op=OP.add)
                    rec = wk.tile([128, ECH, 8], f32, tag="rec", bufs=2)
                    nc.vector.tensor_scalar_add(rec[:], acc[:, :, F:F + 8],
                                                1e-30)
                    nc.vector.reciprocal(rec[:], rec[:])
                    for h in range(H):
                        nc.vector.tensor_tensor(
                            out=acc[:, :, h * fph:(h + 1) * fph],
                            in0=acc[:, :, h * fph:(h + 1) * fph],
                            in1=rec[:, :, h:h + 1].to_broadcast([128, ECH, fph]),
                            op=OP.mult)
                    nc.vector.tensor_tensor(
                        out=acc[:, :, 0:F], in0=acc[:, :, 0:F],
                        in1=b_sb[l][:].rearrange("p (o j) -> p o j", o=1)
                        .to_broadcast([128, ECH, F]),
                        op=OP.add)
                    # ELU: exp(min(x,0)) + max(x,0) - 1
                    t1 = wk.tile([128, ECH, F], f32, tag="et1", bufs=3)
                    nc.vector.tensor_scalar_min(t1[:], acc[:, :, 0:F], 0.0)
                    t2 = wk.tile([128, ECH, F], f32, tag="et1", bufs=3)
                    nc.scalar.activation(t2[:], t1[:], AF.Exp)
                    nc.vector.tensor_scalar_max(acc[:, :, 0:F],
                                                acc[:, :, 0:F], 0.0)
                    nc.vector.tensor_tensor(out=t2[:], in0=t2[:],
                                            in1=acc[:, :, 0:F], op=OP.add)
                    nc.vector.tensor_scalar_add(t2[:], t2[:], -1.0)
                    if l == 2:
                        nc.vector.tensor_tensor(
                            out=t2[:], in0=t2[:],
                            in1=rcnt_sb[:, ch * ECH:(ch + 1) * ECH]
                            .rearrange("p (t o) -> p t o", o=1)
                            .to_broadcast([128, ECH, F]),
                            op=OP.mult)
                    h_bf = wk.tile([128, ECH, F], bf16, tag="hbf",
                                   bufs=C // ECH)
                    nc.vector.tensor_copy(h_bf[:], t2[:])
                    h_bfs.append(h_bf)

                if l < 2:
                    # TABLE PHASE: transpose shard, matmul W', AllGather
                    xT_sb = wk.tile([128, shard], bf16, tag="xt", bufs=1)
                    for c in range(C):
                        pt = ps.tile([128, 128], bf16, tag="pmisc", bufs=2)
                        nc.tensor.transpose(pt[0:F, :],
                                            h_bfs[c // ECH][:, c % ECH, :],
                                            ident_sb[:])
                        nc.vector.tensor_copy(
                            xT_sb[0:F, c * 128:(c + 1) * 128], pt[0:F, :])
                    cin = dram.tile([shard, 256], bf16, tag="cin", bufs=1)
                    alds2 = wk.tile([128, C, 128], bf16, tag="alds", bufs=1)
                    nc.vector.memset(alds2[:], 0.0)
                    for c in range(C):
                        pm = ps.tile([128, 256], f32, tag="pmisc", bufs=2)
                        nc.tensor.matmul(pm[:],
                                         lhsT=xT_sb[0:F, c * 128:(c + 1) * 128],
                                         rhs=w_sb[l + 1][:],
                                         start=True, stop=True)
                        tb = wk.tile([128, 256], bf16, tag="tb", bufs=3)
                        nc.vector.tensor_copy(tb[:], pm[:])
                        nc.vector.tensor_copy(alds2[:, c, 0:8], pm[:, 136:144])
                        nc.sync.dma_start(cin[c * 128:(c + 1) * 128, :], tb[:])
                    nc.sync.dma_start(
                        aldtabs[l + 1].ap()
                        .rearrange("(c p) j -> p c j", p=128), alds2[:])
                    nc.gpsimd.collective_compute(
                        "AllGather", OP.bypass,
                        replica_groups=[list(range(cfg.n_cores))],
                        ins=[cin.opt()], outs=[tables[l + 1].ap()])
                else:
                    # POOLING
                    for grp in range(cfg.n_grp):
                        Sp = wk.tile([128, C, 128], bf16, tag="alds", bufs=1)
                        g3 = grel_sb[grp][:].rearrange("p (t o) -> p t o", o=1)
                        nc.vector.tensor_tensor(
                            out=Sp[:], in0=g3.to_broadcast([128, C, 128]),
                            in1=iota3.to_broadcast([128, C, 128]),
                            op=OP.is_equal)
                        pp = ps.tile([128, 128], f32, tag="stpool", bufs=2)
                        for c in range(C):
                            nc.tensor.matmul(pp[:],
                                             lhsT=h_bfs[c // ECH][:, c % ECH, :],
                                             rhs=Sp[:, c, :],
                                             start=(c == 0), stop=(c == C - 1))
                        nc.vector.tensor_copy(
                            pool_sb[:, grp * 128:(grp + 1) * 128], pp[:])

            # AllReduce pooled sums, then the MLP on every core
            cin2 = dram.tile([128, cfg.n_grp * 128], f32, tag="cin2", bufs=1)
            cred = dram.tile([128, cfg.n_grp * 128], f32, tag="cred", bufs=1)
            nc.sync.dma_start(cin2[:], pool_sb[:])
            nc.gpsimd.collective_compute(
                "AllReduce", OP.add,
                replica_groups=[list(range(cfg.n_cores))],
                ins=[cin2.opt()], outs=[cred.opt()])
            pool2 = wk.tile([128, cfg.n_grp * 128], f32, tag="pool2", bufs=1)
            nc.sync.dma_start(pool2[:], cred[:])
            pa = ps.tile([32, cfg.n_graphs], f32, tag="pmisc", bufs=2)
            nc.tensor.matmul(pa[:], lhsT=fc1_sb[:], rhs=pool2[:, 0:cfg.n_graphs],
                             start=True, stop=True)
            r1 = wk.tile([32, cfg.n_graphs], f32, tag="r1", bufs=1)
            nc.scalar.activation(r1[:], pa[:], AF.Relu, bias=fc1b_sb[:])
            pb = ps.tile([1, cfg.n_graphs], f32, tag="pmisc", bufs=2)
            nc.tensor.matmul(pb[:], lhsT=fc2_sb[:], rhs=r1[:],
                             start=True, stop=True)
            ob = wk.tile([1, cfg.n_graphs], f32, tag="ob", bufs=1)
            nc.scalar.activation(ob[:], pb[:], AF.Identity, bias=fc2b_sb[:])
            nc.sync.dma_start(t_out, ob[:])

    nc.compile()
    return nc


_PROG_CACHE = {}


def run_gat(x, edge_index, batch, weights, cfg, trace=False):
    """weights: dict from make_weights. Returns (out [n_graphs], exec_ns)."""
    from concourse.bass_utils import run_bass_kernel_spmd

    bf = ml_dtypes.bfloat16
    n = cfg.n_nodes
    x_pad = np.zeros((cfg.n_pad, IN_DIM), np.float32)
    x_pad[:n] = np.asarray(x, np.float32)
    xT = np.ascontiguousarray(x_pad.T).astype(bf)

    per_core, _ = preprocess(edge_index, batch, cfg)

    key = cfg.key()
    if key not in _PROG_CACHE:
        _PROG_CACHE[key] = build_program(cfg)
    nc = _PROG_CACHE[key]

    in_maps = []
    for c in range(cfg.n_cores):
        pc = per_core[c]
        m = dict(
            xT=xT,
            xTs=np.ascontiguousarray(xT[:, c * cfg.shard:(c + 1) * cfg.shard]),
            w1p=weights['w1p'], w2p=weights['w2p'], w3p=weights['w3p'],
            b1m=weights['b1m'], b2m=weights['b2m'], b3m=weights['b3m'],
            iota=weights['iota'], ident=weights['ident'],
            idx=pc['idx'], aldix=pc['aldix'], drel=pc['drel'], epi=pc['epi'],
            rcnt=pc['rcnt'],
            fc1=weights['fc1'], fc1b=weights['fc1b'],
            fc2=weights['fc2'], fc2b=weights['fc2b'],
        )
        for g in range(cfg.n_grp):
            m[f"grel{g}"] = pc['grel'][g]
        in_maps.append(m)

    res = run_bass_kernel_spmd(nc, in_maps, core_ids=list(range(cfg.n_cores)),
                               trace=trace)
    out = np.asarray(res.results[0]['out']).reshape(cfg.n_graphs, 1)
    run_gat.last_res = res
    return out, res.exec_time_ns


# ----------------------------------------------------------------------------
# Harness entrypoint: full (unsharded) inputs -> full output [N_GRAPHS, 1].
# Shards edges by destination across the 8 NeuronCores internally.
# ----------------------------------------------------------------------------
_DEF_CFG = None


def kernel(x, edge_index, batch,
           W1, a_src1, a_dst1, b1,
           W2, a_src2, a_dst2, b2,
           W3, a_src3, a_dst3, b3,
           fc1_w, fc1_b, fc2_w, fc2_b):
    global _DEF_CFG
    if _DEF_CFG is None:
        _DEF_CFG = Cfg()  # 50000 nodes / 800000 edges / 256 graphs / 8 cores
    cfg = _DEF_CFG
    weights = make_weights(W1, a_src1, a_dst1, b1, W2, a_src2, a_dst2, b2,
                           W3, a_src3, a_dst3, b3, fc1_w, fc1_b, fc2_w, fc2_b,
                           cfg)
    trace = bool(int(os.environ.get("GAT_BASS_TRACE", "0")))
    out, ns = run_gat(np.asarray(x), np.asarray(edge_index),
                      np.asarray(batch), weights, cfg, trace=trace)
    kernel.exec_time_ns = ns
    return out.astype(np.float32)



# revision 6
# speedup vs baseline: 1.3944x; 1.0252x over previous
"""GAT network (3 GATConv + mean-pool + MLP) as a Bass/Tile SPMD kernel on 8 TRN2 cores.

Sharding: nodes (and edges, partitioned by destination) split into 8 contiguous
node shards. Each core aggregates messages for its shard; bf16 feature tables
(with packed attention logit columns) are AllGathered between layers.

Self-contained: only needs numpy + the container's /opt/trn_rl_repo toolchain.
"""
import sys
import os

sys.path.insert(0, '/opt/trn_rl_repo')

import numpy as np
import ml_dtypes

H = 8
IN_DIM = 16


class Cfg:
    def __init__(self, n_nodes=50000, n_edges=800000, n_graphs=256,
                 n_cores=8, R=56, T=8, G=56, epi_chunk=7):
        self.n_nodes = n_nodes
        self.n_edges = n_edges
        self.n_graphs = n_graphs
        self.n_cores = n_cores
        self.shard = -(-n_nodes // (n_cores * 128)) * 128  # 128-aligned shard
        self.n_pad = self.shard * n_cores
        self.half = self.n_pad // 2
        self.C = self.shard // 128          # node tiles per shard
        self.R = R                          # ranges per pass (uniform)
        self.T = T                          # tiles (of 128 edges) per range
        self.G = G                          # tiles per gather supertile
        self.epi_chunk = epi_chunk          # node-tile cols per epilogue chunk
        assert self.C % epi_chunk == 0
        assert (R * T) % G == 0
        # layer dims
        self.F = [64, 128, 128]             # H * f_out per layer
        self.fph = [8, 16, 16]
        self.elem = [128, 256, 256]         # bf16 row width of gather tables
        self.nrhs = [72, 136, 136]          # msg cols + p cols
        self.PW = [128, 192, 192]           # evac width written to accum
        self.ACCW = 192                     # accum row width (f32, 768B)
        self.n_grp = -(-n_graphs // 128)    # graph groups for pooling

    def key(self):
        return (self.n_pad, self.n_graphs, self.n_cores, self.R, self.T,
                self.G, self.epi_chunk)


def _wrap16(a, reps=8):
    """j -> [j%16, j//16], replicated to 128 partitions."""
    w = a.reshape(-1, 16).T
    return np.ascontiguousarray(np.tile(w, (reps, 1)))


def _wrap128(a):
    """j -> [j%128, j//128]."""
    return np.ascontiguousarray(a.reshape(-1, 128).T)


def preprocess(edge_index, batch, cfg):
    """Build per-core edge-slot arrays. Returns (shared dict, per-core list)."""
    n, npad, shard, half = cfg.n_nodes, cfg.n_pad, cfg.shard, cfg.half
    R, T = cfg.R, cfg.T
    slot_e = T * 128                      # edges per range
    nslots = 2 * R * slot_e               # edge slots per core

    ei = np.asarray(edge_index)
    loops = np.arange(n, dtype=np.int64)
    src = np.concatenate([ei[0], loops])
    dst = np.concatenate([ei[1], loops])
    core = dst // shard

    per_core = []
    max_ranges = 0
    for c in range(cfg.n_cores):
        m = core == c
        srcs = src[m]
        dsts = dst[m] - c * shard
        idx_all = np.zeros(nslots, np.int16)
        aldix_all = np.zeros(2 * R * 128, np.int16)
        drel_all = np.full(nslots, -1.0, np.float32)
        epi = np.zeros(2 * shard, np.int16)
        for pi in range(2):
            pm = (srcs < half) if pi == 0 else (srcs >= half)
            s_p = srcs[pm]
            d_p = dsts[pm]
            order = np.argsort(d_p, kind='stable')
            s_p = s_p[order]
            d_p = d_p[order]
            cnt = np.bincount(d_p, minlength=shard)
            cum = np.concatenate([[0], np.cumsum(cnt)])
            bases = []
            s0 = 0
            while s0 < shard:
                e_node = int(np.searchsorted(cum, cum[s0] + slot_e,
                                             side='right')) - 1
                e_node = min(e_node, s0 + 128)
                assert e_node > s0, f"node {s0} has >{slot_e} edges"
                bases.append(s0)
                s0 = e_node
            nr = len(bases)
            max_ranges = max(max_ranges, nr)
            assert nr <= R, f"need R>={nr}"
            ends = bases[1:] + [shard]
            for r in range(R):
                b0 = bases[r] if r < nr else shard - 1
                aldix_all[(pi * R + r) * 128:(pi * R + r + 1) * 128] = \
                    np.minimum(b0 + np.arange(128), shard - 1)
                if r >= nr:
                    continue
                e0, e1 = int(cum[bases[r]]), int(cum[ends[r]])
                k = e1 - e0
                o = (pi * R + r) * slot_e
                idx_all[o:o + k] = s_p[e0:e1] - pi * half
                drel_all[o:o + k] = d_p[e0:e1] - bases[r]
            # epilogue slot-row index per node
            basearr = np.asarray(bases)
            rix = np.searchsorted(basearr, np.arange(shard), side='right') - 1
            epi[pi * shard:(pi + 1) * shard] = (
                (pi * R + rix) * 128 + (np.arange(shard) - basearr[rix])
            ).astype(np.int16)
        per_core.append(dict(
            idx=_wrap16(idx_all),
            aldix=_wrap16(aldix_all),
            drel=_wrap128(drel_all).astype(ml_dtypes.bfloat16),
            epi=_wrap16(epi),
        ))

    # batch-derived constants
    b = np.asarray(batch)
    cnt_g = np.bincount(b, minlength=cfg.n_graphs).astype(np.float32)
    rcnt_node = np.zeros(npad, np.float32)
    rcnt_node[:n] = 1.0 / np.maximum(cnt_g, 1.0)[b]
    gid = np.full(npad, -1, np.int64)
    gid[:n] = b
    for c in range(cfg.n_cores):
        sl = slice(c * shard, (c + 1) * shard)
        g_loc = gid[sl]
        grels = []
        for grp in range(cfg.n_grp):
            gr = np.where((g_loc >= grp * 128) & (g_loc < (grp + 1) * 128),
                          g_loc - grp * 128, -1).astype(np.float32)
            grels.append(_wrap128(gr).astype(ml_dtypes.bfloat16))
        per_core[c]['grel'] = grels
        per_core[c]['rcnt'] = _wrap128(rcnt_node[sl])
    return per_core, max_ranges


def make_weights(W1, a_src1, a_dst1, b1, W2, a_src2, a_dst2, b2,
                 W3, a_src3, a_dst3, b3, fc1_w, fc1_b, fc2_w, fc2_b, cfg):
    """Host-side weight packing: W' = [W | W@As | W@Ad] per layer."""
    def pack(W, a_s, a_d):
        f = a_s.shape[1]
        As = np.zeros((H * f, H), np.float32)
        Ad = np.zeros((H * f, H), np.float32)
        for h in range(H):
            As[h * f:(h + 1) * f, h] = a_s[h]
            Ad[h * f:(h + 1) * f, h] = a_d[h]
        out = np.concatenate([W, W @ As, W @ Ad], axis=1)
        width = 128 if out.shape[1] <= 128 else 256
        pad = np.zeros((out.shape[0], width - out.shape[1]), np.float32)
        return np.concatenate([out, pad], axis=1)

    bf = ml_dtypes.bfloat16
    sh = dict(
        w1p=pack(np.asarray(W1, np.float32), np.asarray(a_src1), np.asarray(a_dst1)).astype(bf),
        w2p=pack(np.asarray(W2, np.float32), np.asarray(a_src2), np.asarray(a_dst2)).astype(bf),
        w3p=pack(np.asarray(W3, np.float32), np.asarray(a_src3), np.asarray(a_dst3)).astype(bf),
        b1m=np.tile(np.asarray(b1, np.float32)[None, :], (128, 1)),
        b2m=np.tile(np.asarray(b2, np.float32)[None, :], (128, 1)),
        b3m=np.tile(np.asarray(b3, np.float32)[None, :], (128, 1)),
        iota=np.tile(np.arange(128, dtype=np.float32)[None, :], (128, 1)).astype(bf),
        ident=np.eye(128, dtype=np.float32).astype(bf),
        fc1=np.asarray(fc1_w, np.float32),
        fc1b=np.asarray(fc1_b, np.float32).reshape(32, 1),
        fc2=np.asarray(fc2_w, np.float32),
        fc2b=np.asarray(fc2_b, np.float32).reshape(1, 1),
    )
    return sh


def build_program(cfg):
    import concourse.bacc as bacc
    import concourse.bass as bass
    import concourse.tile as tile
    import concourse.mybir as mybir

    f32 = mybir.dt.float32
    bf16 = mybir.dt.bfloat16
    i16 = mybir.dt.int16
    AF = mybir.ActivationFunctionType
    OP = mybir.AluOpType

    npad, shard, C, half = cfg.n_pad, cfg.shard, cfg.C, cfg.half
    R, T, G = cfg.R, cfg.T, cfg.G
    n_sup = 2 * R * T // G          # gather supertiles per layer
    sup_pass = n_sup // 2
    rpg = G // T                    # ranges per supertile
    ECH = cfg.epi_chunk
    ACCW = cfg.ACCW

    nc = bacc.Bacc("TRN2", target_bir_lowering=False, debug=False,
                   num_devices=cfg.n_cores, num_swdge_queues=4)
    _qctr = [0]

    def next_q():
        q = _qctr[0] % 4
        _qctr[0] += 1
        return q

    def din(name, shape, dt):
        return nc.dram_tensor(name, shape, dt, kind="ExternalInput").ap()

    t_xT = din("xT", [IN_DIM, npad], bf16)
    t_xTs = din("xTs", [IN_DIM, shard], bf16)
    t_w = [din("w1p", [IN_DIM, 128], bf16),
           din("w2p", [64, 256], bf16),
           din("w3p", [128, 256], bf16)]
    t_b = [din("b1m", [128, 64], f32),
           din("b2m", [128, 128], f32),
           din("b3m", [128, 128], f32)]
    t_iota = din("iota", [128, 128], bf16)
    t_ident = din("ident", [128, 128], bf16)
    t_idx = din("idx", [128, 2 * R * T * 8], i16)
    t_aldix = din("aldix", [128, 2 * R * 8], i16)
    t_drel = din("drel", [128, 2 * R * T], bf16)
    t_epi = din("epi", [128, 2 * shard // 16], i16)
    t_grel = [din(f"grel{g}", [128, C], bf16) for g in range(cfg.n_grp)]
    t_rcnt = din("rcnt", [128, C], f32)
    t_fc1 = din("fc1", [128, 32], f32)
    t_fc1b = din("fc1b", [32, 1], f32)
    t_fc2 = din("fc2", [32, 1], f32)
    t_fc2b = din("fc2b", [1, 1], f32)
    t_out = nc.dram_tensor("out", [1, cfg.n_graphs], f32,
                           kind="ExternalOutput").ap()

    table1 = [nc.dram_tensor("table1lo", [half, 128], bf16),
              nc.dram_tensor("table1hi", [half, 128], bf16)]
    tables = [None,
              nc.dram_tensor("table2", [npad, 256], bf16, addr_space="Shared"),
              nc.dram_tensor("table3", [npad, 256], bf16, addr_space="Shared")]
    aldtabs = [nc.dram_tensor(f"aldtab{i}", [shard, 128], bf16)
               for i in range(3)]
    accum = nc.dram_tensor("accum", [2 * R * 128, ACCW], f32)

    with tile.TileContext(nc) as tc:
        with tc.tile_pool(name="cst", bufs=1) as cst, \
             tc.tile_pool(name="wk", bufs=1) as wk, \
             tc.tile_pool(name="ps", bufs=1, space="PSUM") as ps, \
             tc.tile_pool(name="dram", bufs=1, space="DRAM") as dram:

            # ---- persistent constants -------------------------------------
            iota_sb = cst.tile([128, 128], bf16)
            nc.sync.dma_start(iota_sb[:], t_iota)
            iota3 = iota_sb[:].rearrange("p (o j) -> p o j", o=1)
            ident_sb = cst.tile([128, 128], bf16)
            nc.sync.dma_start(ident_sb[:], t_ident)
            w_sb = []
            for l in range(3):
                w = cst.tile([t_w[l].shape[0], t_w[l].shape[1]], bf16,
                             name=f"w{l}_sb")
                nc.sync.dma_start(w[:], t_w[l])
                w_sb.append(w)
            b_sb = []
            for l in range(3):
                bt = cst.tile([128, t_b[l].shape[1]], f32, name=f"b{l}_sb")
                nc.sync.dma_start(bt[:], t_b[l])
                b_sb.append(bt)

            grel_sb = []
            for g in range(cfg.n_grp):
                gt_ = cst.tile([128, C], bf16, name=f"grel{g}_sb")
                nc.sync.dma_start(gt_[:], t_grel[g])
                grel_sb.append(gt_)
            rcnt_sb = cst.tile([128, C], f32)
            nc.sync.dma_start(rcnt_sb[:], t_rcnt)
            fc1_sb = cst.tile([128, 32], f32)
            nc.sync.dma_start(fc1_sb[:], t_fc1)
            fc1b_sb = cst.tile([32, 1], f32)
            nc.sync.dma_start(fc1b_sb[:], t_fc1b)
            fc2_sb = cst.tile([32, 1], f32)
            nc.sync.dma_start(fc2_sb[:], t_fc2)
            fc2b_sb = cst.tile([1, 1], f32)
            nc.sync.dma_start(fc2b_sb[:], t_fc2b)

            # ---- L1: every core computes the FULL table1 + its aldtab -----
            xTs_sb = wk.tile([128, shard], bf16, tag="xt", bufs=1)
            nc.sync.dma_start(xTs_sb[0:IN_DIM, :], t_xTs)
            NCHUNK = 512
            for nb in range(npad // NCHUNK):
                xc = wk.tile([IN_DIM, NCHUNK], bf16, tag="xc", bufs=2)
                nc.sync.dma_start(xc[:], t_xT[:, nb * NCHUNK:(nb + 1) * NCHUNK])
                for q in range(NCHUNK // 128):
                    pm = ps.tile([128, 128], f32, tag="pmisc", bufs=2)
                    nc.tensor.matmul(pm[:], lhsT=xc[:, q * 128:(q + 1) * 128],
                                     rhs=w_sb[0][:], start=True, stop=True)
                    hb = wk.tile([128, 128], bf16, tag="tb", bufs=3)
                    nc.vector.tensor_copy(hb[:], pm[:])
                    n0 = nb * NCHUNK + q * 128
                    t1 = table1[0] if n0 < half else table1[1]
                    eng = (nc.sync, nc.scalar)[q % 2]
                    eng.dma_start(t1.ap()[n0 % half:n0 % half + 128, :],
                                  hb[:])
            alds = wk.tile([128, C, 128], bf16, tag="alds", bufs=1)
            nc.vector.memset(alds[:], 0.0)
            for c in range(C):
                pa = ps.tile([128, 8], f32, tag="pmisc", bufs=2)
                nc.tensor.matmul(pa[:], lhsT=xTs_sb[0:IN_DIM, c * 128:(c + 1) * 128],
                                 rhs=w_sb[0][:, 72:80], start=True, stop=True)
                nc.vector.tensor_copy(alds[:, c, 0:8], pa[:])
            nc.sync.dma_start(
                aldtabs[0].ap().rearrange("(c p) j -> p c j", p=128), alds[:])

            pool_sb = cst.tile([128, cfg.n_grp * 128], f32)

            # ---- layers ---------------------------------------------------
            for l in range(3):
                F, fph, elem, nrhs = cfg.F[l], cfg.fph[l], cfg.elem[l], cfg.nrhs[l]
                PW = cfg.PW[l]
                tab = tables[l]

                # EDGE PHASE
                for sp in range(n_sup):
                    pi = 0 if sp < sup_pass else 1
                    if l == 0:
                        tab_ap = table1[pi].ap()
                    else:
                        tab_ap = tab.ap()[pi * half:(pi + 1) * half, :]
                    o8 = sp * G * 8
                    rpsup = G // T
                    idxs = wk.tile([128, G * 8], i16, tag="gidx", bufs=2)
                    nc.sync.dma_start(idxs[:], t_idx[:, o8:o8 + G * 8])
                    aix = wk.tile([128, rpsup * 8], i16, tag="aix", bufs=2)
                    nc.sync.dma_start(
                        aix[:], t_aldix[:, sp * rpsup * 8:(sp + 1) * rpsup * 8])
                    drl = wk.tile([128, G], bf16, tag="drel", bufs=2)
                    nc.sync.dma_start(drl[:], t_drel[:, sp * G:(sp + 1) * G])

                    gt = wk.tile([128, G, elem], bf16, tag="gt", bufs=2)
                    PIECE = 14                    # tiles per gather: 1792 idxs
                    for g0 in range(0, G, PIECE):
                        g1 = min(g0 + PIECE, G)
                        nc.gpsimd.dma_gather(gt[:, g0:g1, :], tab_ap,
                                             idxs[:, g0 * 8:g1 * 8],
                                             num_idxs=(g1 - g0) * 128,
                                             num_idxs_reg=(g1 - g0) * 128,
                                             elem_size=elem,
                                             single_packet=False,
                                             queue_num=next_q())
                    aldr = wk.tile([128, rpsup, 128], bf16, tag="aldr", bufs=2)
                    nc.gpsimd.dma_gather(aldr[:], aldtabs[l].ap(), aix[:],
                                         num_idxs=rpsup * 128,
                                         num_idxs_reg=rpsup * 128,
                                         elem_size=128, single_packet=False,
                                         queue_num=next_q())

                    # expand al_d per edge: per tile, psum_ald = S_T @ aldr
                    aldx = wk.tile([128, G, 8], f32, tag="aldx", bufs=2)
                    Ss = []
                    for rr in range(rpsup):
                        pald = ps.tile([128, T * 8], f32, tag="pmisc", bufs=2)
                        Se = wk.tile([128, T, 128], bf16, tag="S",
                                     bufs=rpsup)
                        d3 = drl[:, rr * T:(rr + 1) * T].rearrange(
                            "p (t o) -> p t o", o=1)
                        nc.vector.tensor_tensor(
                            out=Se[:], in0=d3.to_broadcast([128, T, 128]),
                            in1=iota3.to_broadcast([128, T, 128]),
                            op=OP.is_equal)
                        Ss.append(Se)
                        for t in range(T):
                            pst = ps.tile([128, 128], bf16, tag="stpool", bufs=2)
                            nc.tensor.transpose(pst[:], Se[:, t, :], ident_sb[:])
                            st_sb = wk.tile([128, 128], bf16, tag="st", bufs=3)
                            nc.scalar.activation(st_sb[:], pst[:], AF.Copy)
                            nc.tensor.matmul(pald[:, t * 8:(t + 1) * 8],
                                             lhsT=st_sb[:],
                                             rhs=aldr[:, rr, 0:8],
                                             start=True, stop=True)
                        nc.vector.tensor_copy(aldx[:, rr * T:(rr + 1) * T, :],
                                              pald[:])

                    # p = exp(lrelu(als+ald)) = max(exp(x), exp(0.2x))
                    lg = wk.tile([128, G, 8], f32, tag="lg", bufs=3)
                    nc.vector.tensor_tensor(out=lg[:], in0=gt[:, :, F:F + 8],
                                            in1=aldx[:], op=OP.add)
                    e1 = wk.tile([128, G, 8], f32, tag="lg", bufs=3)
                    nc.scalar.activation(e1[:], lg[:], AF.Exp)
                    e2 = wk.tile([128, G, 8], f32, tag="lg", bufs=3)
                    nc.scalar.activation(e2[:], lg[:], AF.Exp, scale=0.2)
                    nc.vector.tensor_tensor(out=gt[:, :, F:F + 8], in0=e1[:],
                                            in1=e2[:], op=OP.max)
                    # weight messages by p per head
                    for h in range(H):
                        nc.vector.tensor_tensor(
                            out=gt[:, :, h * fph:(h + 1) * fph],
                            in0=gt[:, :, h * fph:(h + 1) * fph],
                            in1=gt[:, :, F + h:F + h + 1].to_broadcast(
                                [128, G, fph]),
                            op=OP.mult)

                    ev = wk.tile([128, rpg, PW], f32, tag="ev", bufs=2)
                    for rr in range(rpg):
                        S = Ss[rr]
                        pacc = ps.tile([128, PW], f32, tag="pacc", bufs=3)
                        for t in range(T):
                            nc.tensor.matmul(pacc[:], lhsT=S[:, t, :],
                                             rhs=gt[:, rr * T + t, 0:PW],
                                             start=(t == 0), stop=(t == T - 1))
                        nc.vector.tensor_copy(ev[:, rr, :], pacc[:])
                    nc.sync.dma_start(
                        accum.ap()[sp * rpg * 128:(sp + 1) * rpg * 128, 0:PW]
                        .rearrange("(s p) w -> p s w", p=128),
                        ev[:])

                # EPILOGUE
                h_bfs = []
                for ch in range(C // ECH):
                    oc = ch * ECH * 8
                    e_lo = wk.tile([128, ECH * 8], i16, tag="ei", bufs=2)
                    nc.sync.dma_start(e_lo[:], t_epi[:, oc:oc + ECH * 8])
                    e_hi = wk.tile([128, ECH * 8], i16, tag="ei2", bufs=2)
                    nc.sync.dma_start(
                        e_hi[:],
                        t_epi[:, shard // 16 + oc:shard // 16 + oc + ECH * 8])
                    glo = wk.tile([128, ECH, PW], f32, tag="eg", bufs=2)
                    nc.gpsimd.dma_gather(glo[:], accum.ap()[:, 0:PW], e_lo[:],
                                         num_idxs=ECH * 128,
                                         num_idxs_reg=ECH * 128,
                                         elem_size=PW, elem_step=ACCW,
                                         single_packet=False,
                                         queue_num=next_q())
                    ghi = wk.tile([128, ECH, PW], f32, tag="eg", bufs=2)
                    nc.gpsimd.dma_gather(ghi[:], accum.ap()[:, 0:PW], e_hi[:],
                                         num_idxs=ECH * 128,
                                         num_idxs_reg=ECH * 128,
                                         elem_size=PW, elem_step=ACCW,
                                         single_packet=False,
                                         queue_num=next_q())
                    acc = wk.tile([128, ECH, nrhs], f32, tag="eacc", bufs=2)
                    nc.vector.tensor_tensor(out=acc[:], in0=glo[:, :, 0:nrhs],
                                            in1=ghi[:, :, 0:nrhs], op=OP.add)
                    rec = wk.tile([128, ECH, 8], f32, tag="rec", bufs=2)
                    nc.vector.tensor_scalar_add(rec[:], acc[:, :, F:F + 8],
                                                1e-30)
                    nc.vector.reciprocal(rec[:], rec[:])
                    for h in range(H):
                        nc.vector.tensor_tensor(
                            out=acc[:, :, h * fph:(h + 1) * fph],
                            in0=acc[:, :, h * fph:(h + 1) * fph],
                            in1=rec[:, :, h:h + 1].to_broadcast([128, ECH, fph]),
                            op=OP.mult)
                    nc.vector.tensor_tensor(
                        out=acc[:, :, 0:F], in0=acc[:, :, 0:F],
                        in1=b_sb[l][:].rearrange("p (o j) -> p o j", o=1)
                        .to_broadcast([128, ECH, F]),
                        op=OP.add)
                    # ELU: exp(min(x,0)) + max(x,0) - 1
                    t1 = wk.tile([128, ECH, F], f32, tag="et1", bufs=3)
                    nc.vector.tensor_scalar_min(t1[:], acc[:, :, 0:F], 0.0)
                    t2 = wk.tile([128, ECH, F], f32, tag="et1", bufs=3)
                    nc.scalar.activation(t2[:], t1[:], AF.Exp)
                    nc.vector.tensor_scalar_max(acc[:, :, 0:F],
                                                acc[:, :, 0:F], 0.0)
                    nc.vector.tensor_tensor(out=t2[:], in0=t2[:],
                                            in1=acc[:, :, 0:F], op=OP.add)
                    nc.vector.tensor_scalar_add(t2[:], t2[:], -1.0)
                    if l == 2:
                        nc.vector.tensor_tensor(
                            out=t2[:], in0=t2[:],
                            in1=rcnt_sb[:, ch * ECH:(ch + 1) * ECH]
                            .rearrange("p (t o) -> p t o", o=1)
                            .to_broadcast([128, ECH, F]),
                            op=OP.mult)
                    h_bf = wk.tile([128, ECH, F], bf16, tag="hbf",
                                   bufs=C // ECH)
                    nc.vector.tensor_copy(h_bf[:], t2[:])
                    h_bfs.append(h_bf)

                if l < 2:
                    # TABLE PHASE: transpose shard, matmul W', AllGather
                    xT_sb = wk.tile([128, shard], bf16, tag="xt", bufs=1)
                    for c in range(C):
                        pt = ps.tile([128, 128], bf16, tag="pmisc", bufs=2)
                        nc.tensor.transpose(pt[0:F, :],
                                            h_bfs[c // ECH][:, c % ECH, :],
                                            ident_sb[:])
                        nc.vector.tensor_copy(
                            xT_sb[0:F, c * 128:(c + 1) * 128], pt[0:F, :])
                    cin = dram.tile([shard, 256], bf16, tag="cin", bufs=1)
                    alds2 = wk.tile([128, C, 128], bf16, tag="alds", bufs=1)
                    nc.vector.memset(alds2[:], 0.0)
                    for c in range(C):
                        pm = ps.tile([128, 256], f32, tag="pmisc", bufs=2)
                        nc.tensor.matmul(pm[:],
                                         lhsT=xT_sb[0:F, c * 128:(c + 1) * 128],
                                         rhs=w_sb[l + 1][:],
                                         start=True, stop=True)
                        tb = wk.tile([128, 256], bf16, tag="tb", bufs=3)
                        nc.vector.tensor_copy(tb[:], pm[:])
                        nc.vector.tensor_copy(alds2[:, c, 0:8], pm[:, 136:144])
                        nc.sync.dma_start(cin[c * 128:(c + 1) * 128, :], tb[:])
                    nc.sync.dma_start(
                        aldtabs[l + 1].ap()
                        .rearrange("(c p) j -> p c j", p=128), alds2[:])
                    nc.gpsimd.collective_compute(
                        "AllGather", OP.bypass,
                        replica_groups=[list(range(cfg.n_cores))],
                        ins=[cin.opt()], outs=[tables[l + 1].ap()])
                else:
                    # POOLING
                    for grp in range(cfg.n_grp):
                        Sp = wk.tile([128, C, 128], bf16, tag="alds", bufs=1)
                        g3 = grel_sb[grp][:].rearrange("p (t o) -> p t o", o=1)
                        nc.vector.tensor_tensor(
                            out=Sp[:], in0=g3.to_broadcast([128, C, 128]),
                            in1=iota3.to_broadcast([128, C, 128]),
                            op=OP.is_equal)
                        pp = ps.tile([128, 128], f32, tag="stpool", bufs=2)
                        for c in range(C):
                            nc.tensor.matmul(pp[:],
                                             lhsT=h_bfs[c // ECH][:, c % ECH, :],
                                             rhs=Sp[:, c, :],
                                             start=(c == 0), stop=(c == C - 1))
                        nc.vector.tensor_copy(
                            pool_sb[:, grp * 128:(grp + 1) * 128], pp[:])

            # AllReduce pooled sums, then the MLP on every core
            cin2 = dram.tile([128, cfg.n_grp * 128], f32, tag="cin2", bufs=1)
            cred = dram.tile([128, cfg.n_grp * 128], f32, tag="cred", bufs=1)
            nc.sync.dma_start(cin2[:], pool_sb[:])
            nc.gpsimd.collective_compute(
                "AllReduce", OP.add,
                replica_groups=[list(range(cfg.n_cores))],
                ins=[cin2.opt()], outs=[cred.opt()])
            pool2 = wk.tile([128, cfg.n_grp * 128], f32, tag="pool2", bufs=1)
            nc.sync.dma_start(pool2[:], cred[:])
            pa = ps.tile([32, cfg.n_graphs], f32, tag="pmisc", bufs=2)
            nc.tensor.matmul(pa[:], lhsT=fc1_sb[:], rhs=pool2[:, 0:cfg.n_graphs],
                             start=True, stop=True)
            r1 = wk.tile([32, cfg.n_graphs], f32, tag="r1", bufs=1)
            nc.scalar.activation(r1[:], pa[:], AF.Relu, bias=fc1b_sb[:])
            pb = ps.tile([1, cfg.n_graphs], f32, tag="pmisc", bufs=2)
            nc.tensor.matmul(pb[:], lhsT=fc2_sb[:], rhs=r1[:],
                             start=True, stop=True)
            ob = wk.tile([1, cfg.n_graphs], f32, tag="ob", bufs=1)
            nc.scalar.activation(ob[:], pb[:], AF.Identity, bias=fc2b_sb[:])
            nc.sync.dma_start(t_out, ob[:])

    nc.compile()
    return nc


_PROG_CACHE = {}


def run_gat(x, edge_index, batch, weights, cfg, trace=False):
    """weights: dict from make_weights. Returns (out [n_graphs], exec_ns)."""
    from concourse.bass_utils import run_bass_kernel_spmd

    bf = ml_dtypes.bfloat16
    n = cfg.n_nodes
    x_pad = np.zeros((cfg.n_pad, IN_DIM), np.float32)
    x_pad[:n] = np.asarray(x, np.float32)
    xT = np.ascontiguousarray(x_pad.T).astype(bf)

    per_core, _ = preprocess(edge_index, batch, cfg)

    key = cfg.key()
    if key not in _PROG_CACHE:
        _PROG_CACHE[key] = build_program(cfg)
    nc = _PROG_CACHE[key]

    in_maps = []
    for c in range(cfg.n_cores):
        pc = per_core[c]
        m = dict(
            xT=xT,
            xTs=np.ascontiguousarray(xT[:, c * cfg.shard:(c + 1) * cfg.shard]),
            w1p=weights['w1p'], w2p=weights['w2p'], w3p=weights['w3p'],
            b1m=weights['b1m'], b2m=weights['b2m'], b3m=weights['b3m'],
            iota=weights['iota'], ident=weights['ident'],
            idx=pc['idx'], aldix=pc['aldix'], drel=pc['drel'], epi=pc['epi'],
            rcnt=pc['rcnt'],
            fc1=weights['fc1'], fc1b=weights['fc1b'],
            fc2=weights['fc2'], fc2b=weights['fc2b'],
        )
        for g in range(cfg.n_grp):
            m[f"grel{g}"] = pc['grel'][g]
        in_maps.append(m)

    res = run_bass_kernel_spmd(nc, in_maps, core_ids=list(range(cfg.n_cores)),
                               trace=trace)
    out = np.asarray(res.results[0]['out']).reshape(cfg.n_graphs, 1)
    run_gat.last_res = res
    return out, res.exec_time_ns


# ----------------------------------------------------------------------------
# Harness entrypoint: full (unsharded) inputs -> full output [N_GRAPHS, 1].
# Shards edges by destination across the 8 NeuronCores internally.
# ----------------------------------------------------------------------------
_DEF_CFG = None


def kernel(x, edge_index, batch,
           W1, a_src1, a_dst1, b1,
           W2, a_src2, a_dst2, b2,
           W3, a_src3, a_dst3, b3,
           fc1_w, fc1_b, fc2_w, fc2_b):
    global _DEF_CFG
    if _DEF_CFG is None:
        _DEF_CFG = Cfg()  # 50000 nodes / 800000 edges / 256 graphs / 8 cores
    cfg = _DEF_CFG
    weights = make_weights(W1, a_src1, a_dst1, b1, W2, a_src2, a_dst2, b2,
                           W3, a_src3, a_dst3, b3, fc1_w, fc1_b, fc2_w, fc2_b,
                           cfg)
    trace = bool(int(os.environ.get("GAT_BASS_TRACE", "0")))
    out, ns = run_gat(np.asarray(x), np.asarray(edge_index),
                      np.asarray(batch), weights, cfg, trace=trace)
    kernel.exec_time_ns = ns
    return out.astype(np.float32)



# revision 15
# speedup vs baseline: 1.6060x; 1.1517x over previous
"""GAT network (3 GATConv + mean-pool + MLP) as a Bass/Tile SPMD kernel on 8 TRN2 cores.

Sharding: nodes (and edges, partitioned by destination) split into 8 contiguous
node shards. Each core aggregates messages for its shard; bf16 feature tables
(with packed attention logit columns) are AllGathered between layers.

Self-contained: only needs numpy + the container's /opt/trn_rl_repo toolchain.
"""
import sys
import os

sys.path.insert(0, '/opt/trn_rl_repo')

import numpy as np
import ml_dtypes

H = 8
IN_DIM = 16


class Cfg:
    def __init__(self, n_nodes=50000, n_edges=800000, n_graphs=256,
                 n_cores=8, R=56, T=8, G=56, epi_chunk=7):
        self.n_nodes = n_nodes
        self.n_edges = n_edges
        self.n_graphs = n_graphs
        self.n_cores = n_cores
        self.shard = -(-n_nodes // (n_cores * 128)) * 128  # 128-aligned shard
        self.n_pad = self.shard * n_cores
        self.half = self.n_pad // 2
        self.C = self.shard // 128          # node tiles per shard
        self.R = R                          # ranges per pass (uniform)
        self.T = T                          # tiles (of 128 edges) per range
        self.G = G                          # tiles per gather supertile
        self.epi_chunk = epi_chunk          # node-tile cols per epilogue chunk
        assert self.C % epi_chunk == 0
        assert (R * T) % G == 0
        # layer dims
        self.F = [64, 128, 128]             # H * f_out per layer
        self.fph = [8, 16, 16]
        self.elem = [128, 256, 256]         # bf16 row width of gather tables
        self.nrhs = [72, 136, 136]          # msg cols + p cols
        self.PW = [128, 192, 192]           # evac width written to accum
        self.ACCW = 192                     # accum row width (f32, 768B)
        self.n_grp = -(-n_graphs // 128)    # graph groups for pooling

    def key(self):
        return (self.n_pad, self.n_graphs, self.n_cores, self.R, self.T,
                self.G, self.epi_chunk)


def _wrap16(a, reps=8):
    """j -> [j%16, j//16], replicated to 128 partitions."""
    w = a.reshape(-1, 16).T
    return np.ascontiguousarray(np.tile(w, (reps, 1)))


def _wrap128(a):
    """j -> [j%128, j//128]."""
    return np.ascontiguousarray(a.reshape(-1, 128).T)


def preprocess(edge_index, batch, cfg):
    """Build per-core edge-slot arrays. Returns (shared dict, per-core list)."""
    n, npad, shard, half = cfg.n_nodes, cfg.n_pad, cfg.shard, cfg.half
    R, T = cfg.R, cfg.T
    slot_e = T * 128                      # edges per range
    nslots = 2 * R * slot_e               # edge slots per core

    ei = np.asarray(edge_index)
    loops = np.arange(n, dtype=np.int64)
    src = np.concatenate([ei[0], loops])
    dst = np.concatenate([ei[1], loops])
    core = dst // shard

    per_core = []
    max_ranges = 0
    for c in range(cfg.n_cores):
        m = core == c
        srcs = src[m]
        dsts = dst[m] - c * shard
        idx_all = np.zeros(nslots, np.int16)
        aldix_all = np.zeros(2 * R * 128, np.int16)
        drel_all = np.full(nslots, -1.0, np.float32)
        epi = np.zeros(2 * shard, np.int16)
        for pi in range(2):
            pm = (srcs < half) if pi == 0 else (srcs >= half)
            s_p = srcs[pm]
            d_p = dsts[pm]
            order = np.argsort(d_p, kind='stable')
            s_p = s_p[order]
            d_p = d_p[order]
            cnt = np.bincount(d_p, minlength=shard)
            cum = np.concatenate([[0], np.cumsum(cnt)])
            bases = []
            s0 = 0
            while s0 < shard:
                e_node = int(np.searchsorted(cum, cum[s0] + slot_e,
                                             side='right')) - 1
                e_node = min(e_node, s0 + 128)
                assert e_node > s0, f"node {s0} has >{slot_e} edges"
                bases.append(s0)
                s0 = e_node
            nr = len(bases)
            max_ranges = max(max_ranges, nr)
            assert nr <= R, f"need R>={nr}"
            ends = bases[1:] + [shard]
            for r in range(R):
                b0 = bases[r] if r < nr else shard - 1
                aldix_all[(pi * R + r) * 128:(pi * R + r + 1) * 128] = \
                    np.minimum(b0 + np.arange(128), shard - 1)
                if r >= nr:
                    continue
                e0, e1 = int(cum[bases[r]]), int(cum[ends[r]])
                k = e1 - e0
                o = (pi * R + r) * slot_e
                idx_all[o:o + k] = s_p[e0:e1] - pi * half
                drel_all[o:o + k] = d_p[e0:e1] - bases[r]
            # epilogue slot-row index per node
            basearr = np.asarray(bases)
            rix = np.searchsorted(basearr, np.arange(shard), side='right') - 1
            epi[pi * shard:(pi + 1) * shard] = (
                (pi * R + rix) * 128 + (np.arange(shard) - basearr[rix])
            ).astype(np.int16)
        per_core.append(dict(
            idx=_wrap16(idx_all),
            aldix=_wrap16(aldix_all),
            drel=_wrap128(drel_all).astype(ml_dtypes.bfloat16),
            epi=_wrap16(epi),
        ))

    # batch-derived constants
    b = np.asarray(batch)
    cnt_g = np.bincount(b, minlength=cfg.n_graphs).astype(np.float32)
    rcnt_node = np.zeros(npad, np.float32)
    rcnt_node[:n] = 1.0 / np.maximum(cnt_g, 1.0)[b]
    gid = np.full(npad, -1, np.int64)
    gid[:n] = b
    for c in range(cfg.n_cores):
        sl = slice(c * shard, (c + 1) * shard)
        g_loc = gid[sl]
        grels = []
        for grp in range(cfg.n_grp):
            gr = np.where((g_loc >= grp * 128) & (g_loc < (grp + 1) * 128),
                          g_loc - grp * 128, -1).astype(np.float32)
            grels.append(_wrap128(gr).astype(ml_dtypes.bfloat16))
        per_core[c]['grel'] = grels
        per_core[c]['rcnt'] = _wrap128(rcnt_node[sl])
    return per_core, max_ranges


USE_PERM = bool(int(os.environ.get("GAT_PERM", "1")))
USE_4D = bool(int(os.environ.get("GAT_4D", "1")))


def _perm(fph):
    """[f][h] interleave: new col f*H+h <- old col h*fph+f."""
    if not USE_PERM:
        return np.arange(fph * H)
    return np.array([(c % H) * fph + c // H for c in range(fph * H)])


def make_weights(W1, a_src1, a_dst1, b1, W2, a_src2, a_dst2, b2,
                 W3, a_src3, a_dst3, b3, fc1_w, fc1_b, fc2_w, fc2_b, cfg):
    """Host-side weight packing: W' = [W@P | W@As | W@Ad] per layer.

    Msg columns are [f][h]-interleaved (head fastest) so the per-head
    broadcasts on-chip keep a packed last dim; each layer's input rows are
    permuted to match the previous layer's output order.
    """
    def pack(W, a_s, a_d):
        f = a_s.shape[1]
        As = np.zeros((H * f, H), np.float32)
        Ad = np.zeros((H * f, H), np.float32)
        for h in range(H):
            As[h * f:(h + 1) * f, h] = a_s[h]
            Ad[h * f:(h + 1) * f, h] = a_d[h]
        out = np.concatenate([W[:, _perm(f)], W @ As, W @ Ad], axis=1)
        width = 128 if out.shape[1] <= 128 else 256
        pad = np.zeros((out.shape[0], width - out.shape[1]), np.float32)
        return np.concatenate([out, pad], axis=1)

    bf = ml_dtypes.bfloat16
    p1, p2, p3 = _perm(8), _perm(16), _perm(16)
    W2r = np.asarray(W2, np.float32)[p1]     # rows follow l1 output order
    W3r = np.asarray(W3, np.float32)[p2]     # rows follow l2 output order
    fc1r = np.asarray(fc1_w, np.float32)[p3]
    sh = dict(
        w1p=pack(np.asarray(W1, np.float32), np.asarray(a_src1), np.asarray(a_dst1)).astype(bf),
        w2p=pack(W2r, np.asarray(a_src2), np.asarray(a_dst2)).astype(bf),
        w3p=pack(W3r, np.asarray(a_src3), np.asarray(a_dst3)).astype(bf),
        b1m=np.tile(np.asarray(b1, np.float32)[_perm(8)][None, :], (128, 1)),
        b2m=np.tile(np.asarray(b2, np.float32)[_perm(16)][None, :], (128, 1)),
        b3m=np.tile(np.asarray(b3, np.float32)[_perm(16)][None, :], (128, 1)),
        iota=np.tile(np.arange(128, dtype=np.float32)[None, :], (128, 1)).astype(bf),
        ident=np.eye(128, dtype=np.float32).astype(bf),
        fc1=np.asarray(fc1_w, np.float32),
        fc1b=np.asarray(fc1_b, np.float32).reshape(32, 1),
        fc2=np.asarray(fc2_w, np.float32),
        fc2b=np.asarray(fc2_b, np.float32).reshape(1, 1),
    )
    return sh


def build_program(cfg):
    import concourse.bacc as bacc
    import concourse.bass as bass
    import concourse.tile as tile
    import concourse.mybir as mybir

    f32 = mybir.dt.float32
    bf16 = mybir.dt.bfloat16
    i16 = mybir.dt.int16
    AF = mybir.ActivationFunctionType
    OP = mybir.AluOpType

    npad, shard, C, half = cfg.n_pad, cfg.shard, cfg.C, cfg.half
    R, T, G = cfg.R, cfg.T, cfg.G
    n_sup = 2 * R * T // G          # gather supertiles per layer
    sup_pass = n_sup // 2
    rpg = G // T                    # ranges per supertile
    ECH = cfg.epi_chunk
    ACCW = cfg.ACCW

    nc = bacc.Bacc("TRN2", target_bir_lowering=False, debug=False,
                   num_devices=cfg.n_cores, num_swdge_queues=4)
    _qctr = [0]

    def next_q():
        q = _qctr[0] % 4
        _qctr[0] += 1
        return q

    def din(name, shape, dt):
        return nc.dram_tensor(name, shape, dt, kind="ExternalInput").ap()

    t_xT = din("xT", [IN_DIM, npad], bf16)
    t_xTs = din("xTs", [IN_DIM, shard], bf16)
    t_w = [din("w1p", [IN_DIM, 128], bf16),
           din("w2p", [64, 256], bf16),
           din("w3p", [128, 256], bf16)]
    t_b = [din("b1m", [128, 64], f32),
           din("b2m", [128, 128], f32),
           din("b3m", [128, 128], f32)]
    t_iota = din("iota", [128, 128], bf16)
    t_ident = din("ident", [128, 128], bf16)
    t_idx = din("idx", [128, 2 * R * T * 8], i16)
    t_aldix = din("aldix", [128, 2 * R * 8], i16)
    t_drel = din("drel", [128, 2 * R * T], bf16)
    t_epi = din("epi", [128, 2 * shard // 16], i16)
    t_grel = [din(f"grel{g}", [128, C], bf16) for g in range(cfg.n_grp)]
    t_rcnt = din("rcnt", [128, C], f32)
    t_fc1 = din("fc1", [128, 32], f32)
    t_fc1b = din("fc1b", [32, 1], f32)
    t_fc2 = din("fc2", [32, 1], f32)
    t_fc2b = din("fc2b", [1, 1], f32)
    t_out = nc.dram_tensor("out", [1, cfg.n_graphs], f32,
                           kind="ExternalOutput").ap()

    table1 = [nc.dram_tensor("table1lo", [half, 128], bf16),
              nc.dram_tensor("table1hi", [half, 128], bf16)]
    tables = [None,
              nc.dram_tensor("table2", [npad, 256], bf16, addr_space="Shared"),
              nc.dram_tensor("table3", [npad, 256], bf16, addr_space="Shared")]
    aldtabs = [nc.dram_tensor(f"aldtab{i}", [shard, 128], bf16)
               for i in range(3)]
    accum = nc.dram_tensor("accum", [2 * R * 128, ACCW], f32)

    with tile.TileContext(nc) as tc:
        with tc.tile_pool(name="cst", bufs=1) as cst, \
             tc.tile_pool(name="wk", bufs=1) as wk, \
             tc.tile_pool(name="ps", bufs=1, space="PSUM") as ps, \
             tc.tile_pool(name="dram", bufs=1, space="DRAM") as dram:

            # ---- persistent constants -------------------------------------
            iota_sb = cst.tile([128, 128], bf16)
            nc.sync.dma_start(iota_sb[:], t_iota)
            iota3 = iota_sb[:].rearrange("p (o j) -> p o j", o=1)
            ident_sb = cst.tile([128, 128], bf16)
            nc.sync.dma_start(ident_sb[:], t_ident)
            w_sb = []
            for l in range(3):
                w = cst.tile([t_w[l].shape[0], t_w[l].shape[1]], bf16,
                             name=f"w{l}_sb")
                nc.sync.dma_start(w[:], t_w[l])
                w_sb.append(w)
            b_sb = []
            for l in range(3):
                bt = cst.tile([128, t_b[l].shape[1]], f32, name=f"b{l}_sb")
                nc.sync.dma_start(bt[:], t_b[l])
                b_sb.append(bt)

            grel_sb = []
            for g in range(cfg.n_grp):
                gt_ = cst.tile([128, C], bf16, name=f"grel{g}_sb")
                nc.sync.dma_start(gt_[:], t_grel[g])
                grel_sb.append(gt_)
            rcnt_sb = cst.tile([128, C], f32)
            nc.sync.dma_start(rcnt_sb[:], t_rcnt)
            fc1_sb = cst.tile([128, 32], f32)
            nc.sync.dma_start(fc1_sb[:], t_fc1)
            fc1b_sb = cst.tile([32, 1], f32)
            nc.sync.dma_start(fc1b_sb[:], t_fc1b)
            fc2_sb = cst.tile([32, 1], f32)
            nc.sync.dma_start(fc2_sb[:], t_fc2)
            fc2b_sb = cst.tile([1, 1], f32)
            nc.sync.dma_start(fc2b_sb[:], t_fc2b)

            # ---- L1: every core computes the FULL table1 + its aldtab -----
            xTs_sb = wk.tile([128, shard], bf16, tag="xt", bufs=1)
            nc.sync.dma_start(xTs_sb[0:IN_DIM, :], t_xTs)
            NCHUNK = 512
            for nb in range(npad // NCHUNK):
                xc = wk.tile([IN_DIM, NCHUNK], bf16, tag="xc", bufs=2)
                nc.sync.dma_start(xc[:], t_xT[:, nb * NCHUNK:(nb + 1) * NCHUNK])
                for q in range(NCHUNK // 128):
                    pm = ps.tile([128, 128], f32, tag="pmisc", bufs=2)
                    nc.tensor.matmul(pm[:], lhsT=xc[:, q * 128:(q + 1) * 128],
                                     rhs=w_sb[0][:], start=True, stop=True)
                    hb = wk.tile([128, 128], bf16, tag="tb", bufs=3)
                    nc.vector.tensor_copy(hb[:], pm[:])
                    n0 = nb * NCHUNK + q * 128
                    t1 = table1[0] if n0 < half else table1[1]
                    eng = (nc.sync, nc.scalar)[q % 2]
                    eng.dma_start(t1.ap()[n0 % half:n0 % half + 128, :],
                                  hb[:])
            alds = wk.tile([128, C, 128], bf16, tag="alds", bufs=1)
            nc.vector.memset(alds[:], 0.0)
            for c in range(C):
                pa = ps.tile([128, 8], f32, tag="pmisc", bufs=2)
                nc.tensor.matmul(pa[:], lhsT=xTs_sb[0:IN_DIM, c * 128:(c + 1) * 128],
                                 rhs=w_sb[0][:, 72:80], start=True, stop=True)
                nc.vector.tensor_copy(alds[:, c, 0:8], pa[:])
            nc.sync.dma_start(
                aldtabs[0].ap().rearrange("(c p) j -> p c j", p=128), alds[:])

            pool_sb = cst.tile([128, cfg.n_grp * 128], f32)

            # ---- layers ---------------------------------------------------
            for l in range(3):
                F, fph, elem, nrhs = cfg.F[l], cfg.fph[l], cfg.elem[l], cfg.nrhs[l]
                PW = cfg.PW[l]
                tab = tables[l]

                # EDGE PHASE
                for sp in range(n_sup):
                    pi = 0 if sp < sup_pass else 1
                    if l == 0:
                        tab_ap = table1[pi].ap()
                    else:
                        tab_ap = tab.ap()[pi * half:(pi + 1) * half, :]
                    o8 = sp * G * 8
                    rpsup = G // T
                    idxs = wk.tile([128, G * 8], i16, tag="gidx", bufs=2)
                    nc.sync.dma_start(idxs[:], t_idx[:, o8:o8 + G * 8])
                    aix = wk.tile([128, rpsup * 8], i16, tag="aix", bufs=2)
                    nc.sync.dma_start(
                        aix[:], t_aldix[:, sp * rpsup * 8:(sp + 1) * rpsup * 8])
                    drl = wk.tile([128, G], bf16, tag="drel", bufs=2)
                    nc.sync.dma_start(drl[:], t_drel[:, sp * G:(sp + 1) * G])

                    gt = wk.tile([128, G, elem], bf16, tag="gt", bufs=2)
                    PIECE = 14                    # tiles per gather: 1792 idxs
                    for g0 in range(0, G, PIECE):
                        g1 = min(g0 + PIECE, G)
                        nc.gpsimd.dma_gather(gt[:, g0:g1, :], tab_ap,
                                             idxs[:, g0 * 8:g1 * 8],
                                             num_idxs=(g1 - g0) * 128,
                                             num_idxs_reg=(g1 - g0) * 128,
                                             elem_size=elem,
                                             single_packet=False,
                                             queue_num=next_q())
                    aldr = wk.tile([128, rpsup, 128], bf16, tag="aldr", bufs=2)
                    nc.gpsimd.dma_gather(aldr[:], aldtabs[l].ap(), aix[:],
                                         num_idxs=rpsup * 128,
                                         num_idxs_reg=rpsup * 128,
                                         elem_size=128, single_packet=False,
                                         queue_num=next_q())

                    # expand al_d per edge: per tile, psum_ald = S_T @ aldr
                    aldx = wk.tile([128, G, 8], f32, tag="aldx", bufs=2)
                    Ss = []
                    for rr in range(rpsup):
                        pald = ps.tile([128, T * 8], f32, tag="pmisc", bufs=2)
                        Se = wk.tile([128, T, 128], bf16, tag="S",
                                     bufs=rpsup)
                        d3 = drl[:, rr * T:(rr + 1) * T].rearrange(
                            "p (t o) -> p t o", o=1)
                        nc.vector.tensor_tensor(
                            out=Se[:], in0=d3.to_broadcast([128, T, 128]),
                            in1=iota3.to_broadcast([128, T, 128]),
                            op=OP.is_equal)
                        Ss.append(Se)
                        pst8 = ps.tile([128, T, 128], bf16, tag="stpool",
                                       bufs=2)
                        for t in range(T):
                            nc.tensor.transpose(pst8[:, t, :], Se[:, t, :],
                                                ident_sb[:])
                        st8 = wk.tile([128, T, 128], bf16, tag="st", bufs=2)
                        nc.scalar.activation(st8[:], pst8[:], AF.Copy)
                        for t in range(T):
                            nc.tensor.matmul(pald[:, t * 8:(t + 1) * 8],
                                             lhsT=st8[:, t, :],
                                             rhs=aldr[:, rr, 0:8],
                                             start=True, stop=True)
                        nc.vector.tensor_copy(aldx[:, rr * T:(rr + 1) * T, :],
                                              pald[:])

                    # p = exp(lrelu(als+ald)) = max(exp(x), exp(0.2x))
                    lg = wk.tile([128, G, 8], f32, tag="lg", bufs=3)
                    nc.vector.tensor_tensor(out=lg[:], in0=gt[:, :, F:F + 8],
                                            in1=aldx[:], op=OP.add)
                    e1 = wk.tile([128, G, 8], f32, tag="lg", bufs=3)
                    nc.scalar.activation(e1[:], lg[:], AF.Exp)
                    e2 = wk.tile([128, G, 8], f32, tag="lg", bufs=3)
                    nc.scalar.activation(e2[:], lg[:], AF.Exp, scale=0.2)
                    nc.vector.tensor_tensor(out=gt[:, :, F:F + 8], in0=e1[:],
                                            in1=e2[:], op=OP.max)
                    # weight messages by p per head ([f][h]-interleaved cols:
                    # head index is the fastest axis, so the broadcast keeps a
                    # packed last dim and the DVE runs in 2x mode)
                    if USE_PERM and USE_4D:
                        gt4 = gt[:, :, 0:F].rearrange("p g (f h) -> p g f h",
                                                      h=H)
                        p4 = gt[:, :, F:F + 8].rearrange(
                            "p g (o h) -> p g o h", o=1)
                        nc.vector.tensor_tensor(
                            out=gt4, in0=gt4,
                            in1=p4.to_broadcast([128, G, fph, H]),
                            op=OP.mult)
                    elif USE_PERM:
                        for f in range(fph):
                            nc.vector.tensor_tensor(
                                out=gt[:, :, f * H:(f + 1) * H],
                                in0=gt[:, :, f * H:(f + 1) * H],
                                in1=gt[:, :, F:F + 8],
                                op=OP.mult)
                    else:
                        for h in range(H):
                            nc.vector.tensor_tensor(
                                out=gt[:, :, h * fph:(h + 1) * fph],
                                in0=gt[:, :, h * fph:(h + 1) * fph],
                                in1=gt[:, :, F + h:F + h + 1].to_broadcast(
                                    [128, G, fph]),
                                op=OP.mult)

                    ev = wk.tile([128, rpg, nrhs], f32, tag="ev", bufs=2)
                    for rr in range(rpg):
                        S = Ss[rr]
                        pacc = ps.tile([128, nrhs], f32, tag="pacc", bufs=3)
                        for t in range(T):
                            nc.tensor.matmul(pacc[:], lhsT=S[:, t, :],
                                             rhs=gt[:, rr * T + t, 0:nrhs],
                                             start=(t == 0), stop=(t == T - 1))
                        nc.vector.tensor_copy(ev[:, rr, :], pacc[:])
                    nc.sync.dma_start(
                        accum.ap()[sp * rpg * 128:(sp + 1) * rpg * 128, 0:nrhs]
                        .rearrange("(s p) w -> p s w", p=128),
                        ev[:])

                # EPILOGUE
                h_bfs = []
                for ch in range(C // ECH):
                    oc = ch * ECH * 8
                    e_lo = wk.tile([128, ECH * 8], i16, tag="ei", bufs=2)
                    nc.sync.dma_start(e_lo[:], t_epi[:, oc:oc + ECH * 8])
                    e_hi = wk.tile([128, ECH * 8], i16, tag="ei2", bufs=2)
                    nc.sync.dma_start(
                        e_hi[:],
                        t_epi[:, shard // 16 + oc:shard // 16 + oc + ECH * 8])
                    glo = wk.tile([128, ECH, PW], f32, tag="eg", bufs=2)
                    nc.gpsimd.dma_gather(glo[:], accum.ap()[:, 0:PW], e_lo[:],
                                         num_idxs=ECH * 128,
                                         num_idxs_reg=ECH * 128,
                                         elem_size=PW, elem_step=ACCW,
                                         single_packet=False,
                                         queue_num=next_q())
                    ghi = wk.tile([128, ECH, PW], f32, tag="eg", bufs=2)
                    nc.gpsimd.dma_gather(ghi[:], accum.ap()[:, 0:PW], e_hi[:],
                                         num_idxs=ECH * 128,
                                         num_idxs_reg=ECH * 128,
                                         elem_size=PW, elem_step=ACCW,
                                         single_packet=False,
                                         queue_num=next_q())
                    acc = wk.tile([128, ECH, nrhs], f32, tag="eacc", bufs=2)
                    nc.vector.tensor_tensor(out=acc[:], in0=glo[:, :, 0:nrhs],
                                            in1=ghi[:, :, 0:nrhs], op=OP.add)
                    rec = wk.tile([128, ECH, 8], f32, tag="rec", bufs=2)
                    nc.vector.tensor_scalar_add(rec[:], acc[:, :, F:F + 8],
                                                1e-30)
                    nc.vector.reciprocal(rec[:], rec[:])
                    if USE_PERM and USE_4D:
                        acc4 = acc[:, :, 0:F].rearrange(
                            "p e (f h) -> p e f h", h=H)
                        rec4 = rec[:].rearrange("p e (o h) -> p e o h", o=1)
                        nc.vector.tensor_tensor(
                            out=acc4, in0=acc4,
                            in1=rec4.to_broadcast([128, ECH, fph, H]),
                            op=OP.mult)
                    elif USE_PERM:
                        for f in range(fph):
                            nc.vector.tensor_tensor(
                                out=acc[:, :, f * H:(f + 1) * H],
                                in0=acc[:, :, f * H:(f + 1) * H],
                                in1=rec[:],
                                op=OP.mult)
                    else:
                        for h in range(H):
                            nc.vector.tensor_tensor(
                                out=acc[:, :, h * fph:(h + 1) * fph],
                                in0=acc[:, :, h * fph:(h + 1) * fph],
                                in1=rec[:, :, h:h + 1].to_broadcast(
                                    [128, ECH, fph]),
                                op=OP.mult)
                    nc.vector.tensor_tensor(
                        out=acc[:, :, 0:F], in0=acc[:, :, 0:F],
                        in1=b_sb[l][:].rearrange("p (o j) -> p o j", o=1)
                        .to_broadcast([128, ECH, F]),
                        op=OP.add)
                    # ELU: exp(min(x,0)) + max(x,0) - 1
                    t1 = wk.tile([128, ECH, F], f32, tag="et1", bufs=3)
                    nc.vector.tensor_scalar_min(t1[:], acc[:, :, 0:F], 0.0)
                    t2 = wk.tile([128, ECH, F], f32, tag="et1", bufs=3)
                    nc.scalar.activation(t2[:], t1[:], AF.Exp)
                    nc.vector.tensor_scalar_max(acc[:, :, 0:F],
                                                acc[:, :, 0:F], 0.0)
                    nc.vector.tensor_tensor(out=t2[:], in0=t2[:],
                                            in1=acc[:, :, 0:F], op=OP.add)
                    nc.vector.tensor_scalar_add(t2[:], t2[:], -1.0)
                    if l == 2:
                        nc.vector.tensor_tensor(
                            out=t2[:], in0=t2[:],
                            in1=rcnt_sb[:, ch * ECH:(ch + 1) * ECH]
                            .rearrange("p (t o) -> p t o", o=1)
                            .to_broadcast([128, ECH, F]),
                            op=OP.mult)
                    h_bf = wk.tile([128, ECH, F], bf16, tag="hbf",
                                   bufs=C // ECH)
                    nc.vector.tensor_copy(h_bf[:], t2[:])
                    h_bfs.append(h_bf)

                if l < 2:
                    # TABLE PHASE: transpose shard, matmul W', AllGather
                    xT_sb = wk.tile([128, shard], bf16, tag="xt", bufs=1)
                    for c in range(C):
                        pt = ps.tile([128, 128], bf16, tag="pmisc", bufs=2)
                        nc.tensor.transpose(pt[0:F, :],
                                            h_bfs[c // ECH][:, c % ECH, :],
                                            ident_sb[:])
                        nc.vector.tensor_copy(
                            xT_sb[0:F, c * 128:(c + 1) * 128], pt[0:F, :])
                    cin = dram.tile([shard, 256], bf16, tag="cin", bufs=1)
                    alds2 = wk.tile([128, C, 128], bf16, tag="alds", bufs=1)
                    nc.vector.memset(alds2[:], 0.0)
                    for c in range(C):
                        pm = ps.tile([128, 256], f32, tag="pmisc", bufs=2)
                        nc.tensor.matmul(pm[:],
                                         lhsT=xT_sb[0:F, c * 128:(c + 1) * 128],
                                         rhs=w_sb[l + 1][:],
                                         start=True, stop=True)
                        tb = wk.tile([128, 256], bf16, tag="tb", bufs=3)
                        nc.vector.tensor_copy(tb[:], pm[:])
                        nc.vector.tensor_copy(alds2[:, c, 0:8], pm[:, 136:144])
                        nc.sync.dma_start(cin[c * 128:(c + 1) * 128, :], tb[:])
                    nc.sync.dma_start(
                        aldtabs[l + 1].ap()
                        .rearrange("(c p) j -> p c j", p=128), alds2[:])
                    nc.gpsimd.collective_compute(
                        "AllGather", OP.bypass,
                        replica_groups=[list(range(cfg.n_cores))],
                        ins=[cin.opt()], outs=[tables[l + 1].ap()])
                else:
                    # POOLING
                    for grp in range(cfg.n_grp):
                        Sp = wk.tile([128, C, 128], bf16, tag="alds", bufs=1)
                        g3 = grel_sb[grp][:].rearrange("p (t o) -> p t o", o=1)
                        nc.vector.tensor_tensor(
                            out=Sp[:], in0=g3.to_broadcast([128, C, 128]),
                            in1=iota3.to_broadcast([128, C, 128]),
                            op=OP.is_equal)
                        pp = ps.tile([128, 128], f32, tag="stpool", bufs=2)
                        for c in range(C):
                            nc.tensor.matmul(pp[:],
                                             lhsT=h_bfs[c // ECH][:, c % ECH, :],
                                             rhs=Sp[:, c, :],
                                             start=(c == 0), stop=(c == C - 1))
                        nc.vector.tensor_copy(
                            pool_sb[:, grp * 128:(grp + 1) * 128], pp[:])

            # AllReduce pooled sums, then the MLP on every core
            cin2 = dram.tile([128, cfg.n_grp * 128], f32, tag="cin2", bufs=1)
            cred = dram.tile([128, cfg.n_grp * 128], f32, tag="cred", bufs=1)
            nc.sync.dma_start(cin2[:], pool_sb[:])
            nc.gpsimd.collective_compute(
                "AllReduce", OP.add,
                replica_groups=[list(range(cfg.n_cores))],
                ins=[cin2.opt()], outs=[cred.opt()])
            pool2 = wk.tile([128, cfg.n_grp * 128], f32, tag="pool2", bufs=1)
            nc.sync.dma_start(pool2[:], cred[:])
            pa = ps.tile([32, cfg.n_graphs], f32, tag="pmisc", bufs=2)
            nc.tensor.matmul(pa[:], lhsT=fc1_sb[:], rhs=pool2[:, 0:cfg.n_graphs],
                             start=True, stop=True)
            r1 = wk.tile([32, cfg.n_graphs], f32, tag="r1", bufs=1)
            nc.scalar.activation(r1[:], pa[:], AF.Relu, bias=fc1b_sb[:])
            pb = ps.tile([1, cfg.n_graphs], f32, tag="pmisc", bufs=2)
            nc.tensor.matmul(pb[:], lhsT=fc2_sb[:], rhs=r1[:],
                             start=True, stop=True)
            ob = wk.tile([1, cfg.n_graphs], f32, tag="ob", bufs=1)
            nc.scalar.activation(ob[:], pb[:], AF.Identity, bias=fc2b_sb[:])
            nc.sync.dma_start(t_out, ob[:])

    nc.compile()
    return nc


_PROG_CACHE = {}


def run_gat(x, edge_index, batch, weights, cfg, trace=False):
    """weights: dict from make_weights. Returns (out [n_graphs], exec_ns)."""
    from concourse.bass_utils import run_bass_kernel_spmd

    bf = ml_dtypes.bfloat16
    n = cfg.n_nodes
    x_pad = np.zeros((cfg.n_pad, IN_DIM), np.float32)
    x_pad[:n] = np.asarray(x, np.float32)
    xT = np.ascontiguousarray(x_pad.T).astype(bf)

    per_core, _ = preprocess(edge_index, batch, cfg)

    key = cfg.key()
    if key not in _PROG_CACHE:
        _PROG_CACHE[key] = build_program(cfg)
    nc = _PROG_CACHE[key]

    in_maps = []
    for c in range(cfg.n_cores):
        pc = per_core[c]
        m = dict(
            xT=xT,
            xTs=np.ascontiguousarray(xT[:, c * cfg.shard:(c + 1) * cfg.shard]),
            w1p=weights['w1p'], w2p=weights['w2p'], w3p=weights['w3p'],
            b1m=weights['b1m'], b2m=weights['b2m'], b3m=weights['b3m'],
            iota=weights['iota'], ident=weights['ident'],
            idx=pc['idx'], aldix=pc['aldix'], drel=pc['drel'], epi=pc['epi'],
            rcnt=pc['rcnt'],
            fc1=weights['fc1'], fc1b=weights['fc1b'],
            fc2=weights['fc2'], fc2b=weights['fc2b'],
        )
        for g in range(cfg.n_grp):
            m[f"grel{g}"] = pc['grel'][g]
        in_maps.append(m)

    res = run_bass_kernel_spmd(nc, in_maps, core_ids=list(range(cfg.n_cores)),
                               trace=trace)
    out = np.asarray(res.results[0]['out']).reshape(cfg.n_graphs, 1)
    run_gat.last_res = res
    return out, res.exec_time_ns


# ----------------------------------------------------------------------------
# Harness entrypoint: full (unsharded) inputs -> full output [N_GRAPHS, 1].
# Shards edges by destination across the 8 NeuronCores internally.
# ----------------------------------------------------------------------------
_DEF_CFG = None


def kernel(x, edge_index, batch,
           W1, a_src1, a_dst1, b1,
           W2, a_src2, a_dst2, b2,
           W3, a_src3, a_dst3, b3,
           fc1_w, fc1_b, fc2_w, fc2_b):
    global _DEF_CFG
    if _DEF_CFG is None:
        _DEF_CFG = Cfg()  # 50000 nodes / 800000 edges / 256 graphs / 8 cores
    cfg = _DEF_CFG
    weights = make_weights(W1, a_src1, a_dst1, b1, W2, a_src2, a_dst2, b2,
                           W3, a_src3, a_dst3, b3, fc1_w, fc1_b, fc2_w, fc2_b,
                           cfg)
    trace = bool(int(os.environ.get("GAT_BASS_TRACE", "0")))
    out, ns = run_gat(np.asarray(x), np.asarray(edge_index),
                      np.asarray(batch), weights, cfg, trace=trace)
    kernel.exec_time_ns = ns
    return out.astype(np.float32)



# revision 20
# speedup vs baseline: 1.6791x; 1.0455x over previous
"""GAT network (3 GATConv + mean-pool + MLP) as a Bass/Tile SPMD kernel on 8 TRN2 cores.

Sharding: nodes (and edges, partitioned by destination) split into 8 contiguous
node shards. Each core aggregates messages for its shard; bf16 feature tables
(with packed attention logit columns) are AllGathered between layers.

Self-contained: only needs numpy + the container's /opt/trn_rl_repo toolchain.
"""
import sys
import os

sys.path.insert(0, '/opt/trn_rl_repo')

import numpy as np
import ml_dtypes

H = 8
IN_DIM = 16


class Cfg:
    def __init__(self, n_nodes=50000, n_edges=800000, n_graphs=256,
                 n_cores=8, R=56, T=8, G=56, epi_chunk=7):
        self.n_nodes = n_nodes
        self.n_edges = n_edges
        self.n_graphs = n_graphs
        self.n_cores = n_cores
        self.shard = -(-n_nodes // (n_cores * 128)) * 128  # 128-aligned shard
        self.n_pad = self.shard * n_cores
        self.half = self.n_pad // 2
        self.C = self.shard // 128          # node tiles per shard
        self.R = R                          # ranges per pass (uniform)
        self.T = T                          # tiles (of 128 edges) per range
        self.G = G                          # tiles per gather supertile
        self.epi_chunk = epi_chunk          # node-tile cols per epilogue chunk
        assert self.C % epi_chunk == 0
        assert (R * T) % G == 0
        # layer dims
        self.F = [64, 128, 128]             # H * f_out per layer
        self.fph = [8, 16, 16]
        self.elem = [128, 256, 256]         # bf16 row width of gather tables
        self.nrhs = [72, 136, 136]          # msg cols + p cols
        self.PW = [128, 192, 192]           # evac width written to accum
        self.ACCW = 192                     # accum row width (f32, 768B)
        self.n_grp = -(-n_graphs // 128)    # graph groups for pooling

    def key(self):
        return (self.n_pad, self.n_graphs, self.n_cores, self.R, self.T,
                self.G, self.epi_chunk)


def _wrap16(a, reps=8):
    """j -> [j%16, j//16], replicated to 128 partitions."""
    w = a.reshape(-1, 16).T
    return np.ascontiguousarray(np.tile(w, (reps, 1)))


def _wrap128(a):
    """j -> [j%128, j//128]."""
    return np.ascontiguousarray(a.reshape(-1, 128).T)


def preprocess(edge_index, batch, cfg):
    """Build per-core edge-slot arrays. Returns (shared dict, per-core list)."""
    n, npad, shard, half = cfg.n_nodes, cfg.n_pad, cfg.shard, cfg.half
    R, T = cfg.R, cfg.T
    slot_e = T * 128                      # edges per range
    nslots = 2 * R * slot_e               # edge slots per core

    ei = np.asarray(edge_index)
    loops = np.arange(n, dtype=np.int64)
    src = np.concatenate([ei[0], loops])
    dst = np.concatenate([ei[1], loops])
    core = dst // shard

    per_core = []
    max_ranges = 0
    for c in range(cfg.n_cores):
        m = core == c
        srcs = src[m]
        dsts = dst[m] - c * shard
        idx_all = np.zeros(nslots, np.int16)
        aldix_all = np.zeros(2 * R * 128, np.int16)
        drel_all = np.full(nslots, -1.0, np.float32)
        epi = np.zeros(2 * shard, np.int16)
        for pi in range(2):
            pm = (srcs < half) if pi == 0 else (srcs >= half)
            s_p = srcs[pm]
            d_p = dsts[pm]
            order = np.argsort(d_p, kind='stable')
            s_p = s_p[order]
            d_p = d_p[order]
            cnt = np.bincount(d_p, minlength=shard)
            cum = np.concatenate([[0], np.cumsum(cnt)])
            bases = []
            s0 = 0
            while s0 < shard:
                e_node = int(np.searchsorted(cum, cum[s0] + slot_e,
                                             side='right')) - 1
                e_node = min(e_node, s0 + 128)
                assert e_node > s0, f"node {s0} has >{slot_e} edges"
                bases.append(s0)
                s0 = e_node
            nr = len(bases)
            max_ranges = max(max_ranges, nr)
            assert nr <= R, f"need R>={nr}"
            ends = bases[1:] + [shard]
            for r in range(R):
                b0 = bases[r] if r < nr else shard - 1
                aldix_all[(pi * R + r) * 128:(pi * R + r + 1) * 128] = \
                    np.minimum(b0 + np.arange(128), shard - 1)
                if r >= nr:
                    continue
                e0, e1 = int(cum[bases[r]]), int(cum[ends[r]])
                k = e1 - e0
                o = (pi * R + r) * slot_e
                idx_all[o:o + k] = s_p[e0:e1] - pi * half
                drel_all[o:o + k] = d_p[e0:e1] - bases[r]
            # epilogue slot-row index per node
            basearr = np.asarray(bases)
            rix = np.searchsorted(basearr, np.arange(shard), side='right') - 1
            epi[pi * shard:(pi + 1) * shard] = (
                (pi * R + rix) * 128 + (np.arange(shard) - basearr[rix])
            ).astype(np.int16)
        per_core.append(dict(
            idx=_wrap16(idx_all),
            aldix=_wrap16(aldix_all),
            drel=_wrap128(drel_all).astype(ml_dtypes.bfloat16),
            epi=_wrap16(epi),
        ))

    # batch-derived constants
    b = np.asarray(batch)
    cnt_g = np.bincount(b, minlength=cfg.n_graphs).astype(np.float32)
    rcnt_node = np.zeros(npad, np.float32)
    rcnt_node[:n] = 1.0 / np.maximum(cnt_g, 1.0)[b]
    gid = np.full(npad, -1, np.int64)
    gid[:n] = b
    for c in range(cfg.n_cores):
        sl = slice(c * shard, (c + 1) * shard)
        g_loc = gid[sl]
        grels = []
        for grp in range(cfg.n_grp):
            gr = np.where((g_loc >= grp * 128) & (g_loc < (grp + 1) * 128),
                          g_loc - grp * 128, -1).astype(np.float32)
            grels.append(_wrap128(gr).astype(ml_dtypes.bfloat16))
        per_core[c]['grel'] = grels
        per_core[c]['rcnt'] = _wrap128(rcnt_node[sl])
    return per_core, max_ranges


USE_PERM = bool(int(os.environ.get("GAT_PERM", "1")))
USE_4D = bool(int(os.environ.get("GAT_4D", "1")))


def _perm(fph):
    """[f][h] interleave: new col f*H+h <- old col h*fph+f."""
    if not USE_PERM:
        return np.arange(fph * H)
    return np.array([(c % H) * fph + c // H for c in range(fph * H)])


def make_weights(W1, a_src1, a_dst1, b1, W2, a_src2, a_dst2, b2,
                 W3, a_src3, a_dst3, b3, fc1_w, fc1_b, fc2_w, fc2_b, cfg):
    """Host-side weight packing: W' = [W@P | W@As | W@Ad] per layer.

    Msg columns are [f][h]-interleaved (head fastest) so the per-head
    broadcasts on-chip keep a packed last dim; each layer's input rows are
    permuted to match the previous layer's output order.
    """
    def pack(W, a_s, a_d):
        f = a_s.shape[1]
        As = np.zeros((H * f, H), np.float32)
        Ad = np.zeros((H * f, H), np.float32)
        for h in range(H):
            As[h * f:(h + 1) * f, h] = a_s[h]
            Ad[h * f:(h + 1) * f, h] = a_d[h]
        out = np.concatenate([W[:, _perm(f)], W @ As, W @ Ad], axis=1)
        width = 128 if out.shape[1] <= 128 else 256
        pad = np.zeros((out.shape[0], width - out.shape[1]), np.float32)
        return np.concatenate([out, pad], axis=1)

    bf = ml_dtypes.bfloat16
    p1, p2, p3 = _perm(8), _perm(16), _perm(16)
    W2r = np.asarray(W2, np.float32)[p1]     # rows follow l1 output order
    W3r = np.asarray(W3, np.float32)[p2]     # rows follow l2 output order
    fc1r = np.asarray(fc1_w, np.float32)[p3]
    sh = dict(
        w1p=pack(np.asarray(W1, np.float32), np.asarray(a_src1), np.asarray(a_dst1)).astype(bf),
        w2p=pack(W2r, np.asarray(a_src2), np.asarray(a_dst2)).astype(bf),
        w3p=pack(W3r, np.asarray(a_src3), np.asarray(a_dst3)).astype(bf),
        b1m=np.tile(np.asarray(b1, np.float32)[_perm(8)][None, :], (128, 1)),
        b2m=np.tile(np.asarray(b2, np.float32)[_perm(16)][None, :], (128, 1)),
        b3m=np.tile(np.asarray(b3, np.float32)[_perm(16)][None, :], (128, 1)),
        iota=np.tile(np.arange(128, dtype=np.float32)[None, :], (128, 1)).astype(bf),
        ident=np.eye(128, dtype=np.float32).astype(bf),
        fc1=fc1r,
        fc1b=np.asarray(fc1_b, np.float32).reshape(32, 1),
        fc2=np.asarray(fc2_w, np.float32),
        fc2b=np.asarray(fc2_b, np.float32).reshape(1, 1),
    )
    return sh


def build_program(cfg):
    import concourse.bacc as bacc
    import concourse.bass as bass
    import concourse.tile as tile
    import concourse.mybir as mybir

    f32 = mybir.dt.float32
    bf16 = mybir.dt.bfloat16
    i16 = mybir.dt.int16
    AF = mybir.ActivationFunctionType
    OP = mybir.AluOpType

    npad, shard, C, half = cfg.n_pad, cfg.shard, cfg.C, cfg.half
    R, T, G = cfg.R, cfg.T, cfg.G
    n_sup = 2 * R * T // G          # gather supertiles per layer
    sup_pass = n_sup // 2
    rpg = G // T                    # ranges per supertile
    ECH = cfg.epi_chunk
    ACCW = cfg.ACCW

    nc = bacc.Bacc("TRN2", target_bir_lowering=False, debug=False,
                   num_devices=cfg.n_cores, num_swdge_queues=4)
    # NOTE: each tile buffer's DMA-completion semaphore is locked to one SWDGE
    # queue, so queue assignment must be deterministic per buffer/region.

    def din(name, shape, dt):
        return nc.dram_tensor(name, shape, dt, kind="ExternalInput").ap()

    t_xT = din("xT", [IN_DIM, npad], bf16)
    t_xTs = din("xTs", [IN_DIM, shard], bf16)
    t_w = [din("w1p", [IN_DIM, 128], bf16),
           din("w2p", [64, 256], bf16),
           din("w3p", [128, 256], bf16)]
    t_b = [din("b1m", [128, 64], f32),
           din("b2m", [128, 128], f32),
           din("b3m", [128, 128], f32)]
    t_iota = din("iota", [128, 128], bf16)
    t_ident = din("ident", [128, 128], bf16)
    t_idx = din("idx", [128, 2 * R * T * 8], i16)
    t_aldix = din("aldix", [128, 2 * R * 8], i16)
    t_drel = din("drel", [128, 2 * R * T], bf16)
    t_epi = din("epi", [128, 2 * shard // 16], i16)
    t_grel = [din(f"grel{g}", [128, C], bf16) for g in range(cfg.n_grp)]
    t_rcnt = din("rcnt", [128, C], f32)
    t_fc1 = din("fc1", [128, 32], f32)
    t_fc1b = din("fc1b", [32, 1], f32)
    t_fc2 = din("fc2", [32, 1], f32)
    t_fc2b = din("fc2b", [1, 1], f32)
    t_out = nc.dram_tensor("out", [1, cfg.n_graphs], f32,
                           kind="ExternalOutput").ap()

    table1 = [nc.dram_tensor("table1lo", [half, 128], bf16),
              nc.dram_tensor("table1hi", [half, 128], bf16)]
    tables = [None,
              nc.dram_tensor("table2", [npad, 256], bf16, addr_space="Shared"),
              nc.dram_tensor("table3", [npad, 256], bf16, addr_space="Shared")]
    aldtabs = [nc.dram_tensor(f"aldtab{i}", [shard, 128], bf16)
               for i in range(3)]
    accum = nc.dram_tensor("accum", [2 * R * 128, ACCW], f32)

    with tile.TileContext(nc) as tc:
        with tc.tile_pool(name="cst", bufs=1) as cst, \
             tc.tile_pool(name="wk", bufs=1) as wk, \
             tc.tile_pool(name="ps", bufs=1, space="PSUM") as ps, \
             tc.tile_pool(name="dram", bufs=1, space="DRAM") as dram:

            # ---- persistent constants -------------------------------------
            iota_sb = cst.tile([128, 128], bf16)
            nc.sync.dma_start(iota_sb[:], t_iota)
            iota3 = iota_sb[:].rearrange("p (o j) -> p o j", o=1)
            ident_sb = cst.tile([128, 128], bf16)
            nc.sync.dma_start(ident_sb[:], t_ident)
            w_sb = []
            for l in range(3):
                w = cst.tile([t_w[l].shape[0], t_w[l].shape[1]], bf16,
                             name=f"w{l}_sb")
                nc.sync.dma_start(w[:], t_w[l])
                w_sb.append(w)
            b_sb = []
            for l in range(3):
                bt = cst.tile([128, t_b[l].shape[1]], f32, name=f"b{l}_sb")
                nc.sync.dma_start(bt[:], t_b[l])
                b_sb.append(bt)

            grel_sb = []
            for g in range(cfg.n_grp):
                gt_ = cst.tile([128, C], bf16, name=f"grel{g}_sb")
                nc.sync.dma_start(gt_[:], t_grel[g])
                grel_sb.append(gt_)
            rcnt_sb = cst.tile([128, C], f32)
            nc.sync.dma_start(rcnt_sb[:], t_rcnt)
            fc1_sb = cst.tile([128, 32], f32)
            nc.sync.dma_start(fc1_sb[:], t_fc1)
            fc1b_sb = cst.tile([32, 1], f32)
            nc.sync.dma_start(fc1b_sb[:], t_fc1b)
            fc2_sb = cst.tile([32, 1], f32)
            nc.sync.dma_start(fc2_sb[:], t_fc2)
            fc2b_sb = cst.tile([1, 1], f32)
            nc.sync.dma_start(fc2b_sb[:], t_fc2b)

            # ---- L1: every core computes the FULL table1 + its aldtab -----
            xTs_sb = wk.tile([128, shard], bf16, tag="xt", bufs=1)
            nc.sync.dma_start(xTs_sb[0:IN_DIM, :], t_xTs)
            NCHUNK = 512
            for nb in range(npad // NCHUNK):
                xc = wk.tile([IN_DIM, NCHUNK], bf16, tag="xc", bufs=2)
                nc.sync.dma_start(xc[:], t_xT[:, nb * NCHUNK:(nb + 1) * NCHUNK])
                for q in range(NCHUNK // 128):
                    pm = ps.tile([128, 128], f32, tag="pmisc", bufs=2)
                    nc.tensor.matmul(pm[:], lhsT=xc[:, q * 128:(q + 1) * 128],
                                     rhs=w_sb[0][:], start=True, stop=True)
                    hb = wk.tile([128, 128], bf16, tag="tb", bufs=3)
                    nc.vector.tensor_copy(hb[:], pm[:])
                    n0 = nb * NCHUNK + q * 128
                    t1 = table1[0] if n0 < half else table1[1]
                    eng = (nc.sync, nc.scalar)[q % 2]
                    eng.dma_start(t1.ap()[n0 % half:n0 % half + 128, :],
                                  hb[:])
            alds = wk.tile([128, C, 128], bf16, tag="alds", bufs=1)
            nc.vector.memset(alds[:], 0.0)
            for c in range(C):
                pa = ps.tile([128, 8], f32, tag="pmisc", bufs=2)
                nc.tensor.matmul(pa[:], lhsT=xTs_sb[0:IN_DIM, c * 128:(c + 1) * 128],
                                 rhs=w_sb[0][:, 72:80], start=True, stop=True)
                nc.vector.tensor_copy(alds[:, c, 0:8], pa[:])
            nc.sync.dma_start(
                aldtabs[0].ap().rearrange("(c p) j -> p c j", p=128), alds[:])

            pool_sb = cst.tile([128, cfg.n_grp * 128], f32)

            # ---- layers ---------------------------------------------------
            for l in range(3):
                F, fph, elem, nrhs = cfg.F[l], cfg.fph[l], cfg.elem[l], cfg.nrhs[l]
                PW = cfg.PW[l]
                tab = tables[l]

                # EDGE PHASE
                for sp in range(n_sup):
                    pi = 0 if sp < sup_pass else 1
                    if l == 0:
                        tab_ap = table1[pi].ap()
                    else:
                        tab_ap = tab.ap()[pi * half:(pi + 1) * half, :]
                    o8 = sp * G * 8
                    rpsup = G // T
                    idxs = wk.tile([128, G * 8], i16, tag="gidx", bufs=2)
                    nc.sync.dma_start(idxs[:], t_idx[:, o8:o8 + G * 8])
                    aix = wk.tile([128, rpsup * 8], i16, tag="aix", bufs=2)
                    nc.sync.dma_start(
                        aix[:], t_aldix[:, sp * rpsup * 8:(sp + 1) * rpsup * 8])
                    drl = wk.tile([128, G], bf16, tag="drel", bufs=2)
                    nc.sync.dma_start(drl[:], t_drel[:, sp * G:(sp + 1) * G])

                    gt = wk.tile([128, G, elem], bf16, tag="gt", bufs=2)
                    PIECE = 14                    # tiles per gather: 1792 idxs
                    for g0 in range(0, G, PIECE):
                        g1 = min(g0 + PIECE, G)
                        nc.gpsimd.dma_gather(gt[:, g0:g1, :], tab_ap,
                                             idxs[:, g0 * 8:g1 * 8],
                                             num_idxs=(g1 - g0) * 128,
                                             num_idxs_reg=(g1 - g0) * 128,
                                             elem_size=elem,
                                             single_packet=False,
                                             queue_num=(g0 // PIECE) % 4)
                    aldr = wk.tile([128, rpsup, 128], bf16, tag="aldr", bufs=2)
                    nc.gpsimd.dma_gather(aldr[:], aldtabs[l].ap(), aix[:],
                                         num_idxs=rpsup * 128,
                                         num_idxs_reg=rpsup * 128,
                                         elem_size=128, single_packet=False,
                                         queue_num=sp % 2)

                    # expand al_d per edge: per tile, psum_ald = S_T @ aldr
                    aldx = wk.tile([128, G, 8], f32, tag="aldx", bufs=2)
                    Ss = []
                    for rr in range(rpsup):
                        pald = ps.tile([128, T * 8], f32, tag="pmisc", bufs=2)
                        Se = wk.tile([128, T, 128], bf16, tag="S",
                                     bufs=rpsup)
                        d3 = drl[:, rr * T:(rr + 1) * T].rearrange(
                            "p (t o) -> p t o", o=1)
                        nc.vector.tensor_tensor(
                            out=Se[:], in0=d3.to_broadcast([128, T, 128]),
                            in1=iota3.to_broadcast([128, T, 128]),
                            op=OP.is_equal)
                        Ss.append(Se)
                        pst8 = ps.tile([128, T, 128], bf16, tag="stpool",
                                       bufs=2)
                        for t in range(T):
                            nc.tensor.transpose(pst8[:, t, :], Se[:, t, :],
                                                ident_sb[:])
                        st8 = wk.tile([128, T, 128], bf16, tag="st", bufs=2)
                        nc.scalar.activation(st8[:], pst8[:], AF.Copy)
                        for t in range(T):
                            nc.tensor.matmul(pald[:, t * 8:(t + 1) * 8],
                                             lhsT=st8[:, t, :],
                                             rhs=aldr[:, rr, 0:8],
                                             start=True, stop=True)
                        nc.vector.tensor_copy(aldx[:, rr * T:(rr + 1) * T, :],
                                              pald[:])

                    # p = exp(lrelu(als+ald)) = max(exp(x), exp(0.2x))
                    lg = wk.tile([128, G, 8], f32, tag="lg", bufs=3)
                    nc.vector.tensor_tensor(out=lg[:], in0=gt[:, :, F:F + 8],
                                            in1=aldx[:], op=OP.add)
                    e1 = wk.tile([128, G, 8], f32, tag="lg", bufs=3)
                    nc.scalar.activation(e1[:], lg[:], AF.Exp)
                    e2 = wk.tile([128, G, 8], f32, tag="lg", bufs=3)
                    nc.scalar.activation(e2[:], lg[:], AF.Exp, scale=0.2)
                    nc.vector.tensor_tensor(out=gt[:, :, F:F + 8], in0=e1[:],
                                            in1=e2[:], op=OP.max)
                    # weight messages by p per head ([f][h]-interleaved cols:
                    # head index is the fastest axis, so the broadcast keeps a
                    # packed last dim and the DVE runs in 2x mode)
                    if USE_PERM and USE_4D:
                        gt4 = gt[:, :, 0:F].rearrange("p g (f h) -> p g f h",
                                                      h=H)
                        p4 = gt[:, :, F:F + 8].rearrange(
                            "p g (o h) -> p g o h", o=1)
                        nc.vector.tensor_tensor(
                            out=gt4, in0=gt4,
                            in1=p4.to_broadcast([128, G, fph, H]),
                            op=OP.mult)
                    elif USE_PERM:
                        for f in range(fph):
                            nc.vector.tensor_tensor(
                                out=gt[:, :, f * H:(f + 1) * H],
                                in0=gt[:, :, f * H:(f + 1) * H],
                                in1=gt[:, :, F:F + 8],
                                op=OP.mult)
                    else:
                        for h in range(H):
                            nc.vector.tensor_tensor(
                                out=gt[:, :, h * fph:(h + 1) * fph],
                                in0=gt[:, :, h * fph:(h + 1) * fph],
                                in1=gt[:, :, F + h:F + h + 1].to_broadcast(
                                    [128, G, fph]),
                                op=OP.mult)

                    ev = wk.tile([128, rpg, nrhs], f32, tag="ev", bufs=2)
                    for rr in range(rpg):
                        S = Ss[rr]
                        pacc = ps.tile([128, nrhs], f32, tag="pacc", bufs=3)
                        for t in range(T):
                            nc.tensor.matmul(pacc[:], lhsT=S[:, t, :],
                                             rhs=gt[:, rr * T + t, 0:nrhs],
                                             start=(t == 0), stop=(t == T - 1))
                        nc.vector.tensor_copy(ev[:, rr, :], pacc[:])
                    nc.sync.dma_start(
                        accum.ap()[sp * rpg * 128:(sp + 1) * rpg * 128, 0:nrhs]
                        .rearrange("(s p) w -> p s w", p=128),
                        ev[:])

                # EPILOGUE
                h_bfs = []
                for ch in range(C // ECH):
                    oc = ch * ECH * 8
                    e_lo = wk.tile([128, ECH * 8], i16, tag="ei", bufs=2)
                    nc.sync.dma_start(e_lo[:], t_epi[:, oc:oc + ECH * 8])
                    e_hi = wk.tile([128, ECH * 8], i16, tag="ei2", bufs=2)
                    nc.sync.dma_start(
                        e_hi[:],
                        t_epi[:, shard // 16 + oc:shard // 16 + oc + ECH * 8])
                    glo = wk.tile([128, ECH, PW], f32, tag="eg", bufs=2)
                    nc.gpsimd.dma_gather(glo[:], accum.ap()[:, 0:PW], e_lo[:],
                                         num_idxs=ECH * 128,
                                         num_idxs_reg=ECH * 128,
                                         elem_size=PW, elem_step=ACCW,
                                         single_packet=False, queue_num=2)
                    ghi = wk.tile([128, ECH, PW], f32, tag="eg", bufs=2)
                    nc.gpsimd.dma_gather(ghi[:], accum.ap()[:, 0:PW], e_hi[:],
                                         num_idxs=ECH * 128,
                                         num_idxs_reg=ECH * 128,
                                         elem_size=PW, elem_step=ACCW,
                                         single_packet=False, queue_num=3)
                    acc = wk.tile([128, ECH, nrhs], f32, tag="eacc", bufs=2)
                    nc.vector.tensor_tensor(out=acc[:], in0=glo[:, :, 0:nrhs],
                                            in1=ghi[:, :, 0:nrhs], op=OP.add)
                    rec = wk.tile([128, ECH, 8], f32, tag="rec", bufs=2)
                    nc.vector.tensor_scalar_add(rec[:], acc[:, :, F:F + 8],
                                                1e-30)
                    nc.vector.reciprocal(rec[:], rec[:])
                    if USE_PERM and USE_4D:
                        acc4 = acc[:, :, 0:F].rearrange(
                            "p e (f h) -> p e f h", h=H)
                        rec4 = rec[:].rearrange("p e (o h) -> p e o h", o=1)
                        nc.vector.tensor_tensor(
                            out=acc4, in0=acc4,
                            in1=rec4.to_broadcast([128, ECH, fph, H]),
                            op=OP.mult)
                    elif USE_PERM:
                        for f in range(fph):
                            nc.vector.tensor_tensor(
                                out=acc[:, :, f * H:(f + 1) * H],
                                in0=acc[:, :, f * H:(f + 1) * H],
                                in1=rec[:],
                                op=OP.mult)
                    else:
                        for h in range(H):
                            nc.vector.tensor_tensor(
                                out=acc[:, :, h * fph:(h + 1) * fph],
                                in0=acc[:, :, h * fph:(h + 1) * fph],
                                in1=rec[:, :, h:h + 1].to_broadcast(
                                    [128, ECH, fph]),
                                op=OP.mult)
                    nc.vector.tensor_tensor(
                        out=acc[:, :, 0:F], in0=acc[:, :, 0:F],
                        in1=b_sb[l][:].rearrange("p (o j) -> p o j", o=1)
                        .to_broadcast([128, ECH, F]),
                        op=OP.add)
                    # ELU: exp(min(x,0)) + max(x,0) - 1
                    t1 = wk.tile([128, ECH, F], f32, tag="et1", bufs=3)
                    nc.vector.tensor_scalar_min(t1[:], acc[:, :, 0:F], 0.0)
                    t2 = wk.tile([128, ECH, F], f32, tag="et1", bufs=3)
                    nc.scalar.activation(t2[:], t1[:], AF.Exp)
                    nc.vector.tensor_scalar_max(acc[:, :, 0:F],
                                                acc[:, :, 0:F], 0.0)
                    nc.vector.tensor_tensor(out=t2[:], in0=t2[:],
                                            in1=acc[:, :, 0:F], op=OP.add)
                    nc.vector.tensor_scalar_add(t2[:], t2[:], -1.0)
                    if l == 2:
                        nc.vector.tensor_tensor(
                            out=t2[:], in0=t2[:],
                            in1=rcnt_sb[:, ch * ECH:(ch + 1) * ECH]
                            .rearrange("p (t o) -> p t o", o=1)
                            .to_broadcast([128, ECH, F]),
                            op=OP.mult)
                    h_bf = wk.tile([128, ECH, F], bf16, tag="hbf",
                                   bufs=C // ECH)
                    nc.vector.tensor_copy(h_bf[:], t2[:])
                    h_bfs.append(h_bf)

                if l < 2:
                    # TABLE PHASE: transpose shard, matmul W', AllGather
                    xT_sb = wk.tile([128, shard], bf16, tag="xt", bufs=1)
                    for c in range(C):
                        pt = ps.tile([128, 128], bf16, tag="pmisc", bufs=2)
                        nc.tensor.transpose(pt[0:F, :],
                                            h_bfs[c // ECH][:, c % ECH, :],
                                            ident_sb[:])
                        nc.vector.tensor_copy(
                            xT_sb[0:F, c * 128:(c + 1) * 128], pt[0:F, :])
                    cin = dram.tile([shard, 256], bf16, tag="cin", bufs=1)
                    alds2 = wk.tile([128, C, 128], bf16, tag="alds", bufs=1)
                    nc.vector.memset(alds2[:], 0.0)
                    for c in range(C):
                        pm = ps.tile([128, 256], f32, tag="pmisc", bufs=2)
                        nc.tensor.matmul(pm[:],
                                         lhsT=xT_sb[0:F, c * 128:(c + 1) * 128],
                                         rhs=w_sb[l + 1][:],
                                         start=True, stop=True)
                        tb = wk.tile([128, 256], bf16, tag="tb", bufs=3)
                        nc.vector.tensor_copy(tb[:], pm[:])
                        nc.vector.tensor_copy(alds2[:, c, 0:8], pm[:, 136:144])
                        nc.sync.dma_start(cin[c * 128:(c + 1) * 128, :], tb[:])
                    nc.sync.dma_start(
                        aldtabs[l + 1].ap()
                        .rearrange("(c p) j -> p c j", p=128), alds2[:])
                    nc.gpsimd.collective_compute(
                        "AllGather", OP.bypass,
                        replica_groups=[list(range(cfg.n_cores))],
                        ins=[cin.opt()], outs=[tables[l + 1].ap()])
                else:
                    # POOLING
                    for grp in range(cfg.n_grp):
                        Sp = wk.tile([128, C, 128], bf16, tag="alds", bufs=1)
                        g3 = grel_sb[grp][:].rearrange("p (t o) -> p t o", o=1)
                        nc.vector.tensor_tensor(
                            out=Sp[:], in0=g3.to_broadcast([128, C, 128]),
                            in1=iota3.to_broadcast([128, C, 128]),
                            op=OP.is_equal)
                        pp = ps.tile([128, 128], f32, tag="stpool", bufs=2)
                        for c in range(C):
                            nc.tensor.matmul(pp[:],
                                             lhsT=h_bfs[c // ECH][:, c % ECH, :],
                                             rhs=Sp[:, c, :],
                                             start=(c == 0), stop=(c == C - 1))
                        nc.vector.tensor_copy(
                            pool_sb[:, grp * 128:(grp + 1) * 128], pp[:])

            # AllReduce pooled sums, then the MLP on every core
            cin2 = dram.tile([128, cfg.n_grp * 128], f32, tag="cin2", bufs=1)
            cred = dram.tile([128, cfg.n_grp * 128], f32, tag="cred", bufs=1)
            nc.sync.dma_start(cin2[:], pool_sb[:])
            nc.gpsimd.collective_compute(
                "AllReduce", OP.add,
                replica_groups=[list(range(cfg.n_cores))],
                ins=[cin2.opt()], outs=[cred.opt()])
            pool2 = wk.tile([128, cfg.n_grp * 128], f32, tag="pool2", bufs=1)
            nc.sync.dma_start(pool2[:], cred[:])
            pa = ps.tile([32, cfg.n_graphs], f32, tag="pmisc", bufs=2)
            nc.tensor.matmul(pa[:], lhsT=fc1_sb[:], rhs=pool2[:, 0:cfg.n_graphs],
                             start=True, stop=True)
            r1 = wk.tile([32, cfg.n_graphs], f32, tag="r1", bufs=1)
            nc.scalar.activation(r1[:], pa[:], AF.Relu, bias=fc1b_sb[:])
            pb = ps.tile([1, cfg.n_graphs], f32, tag="pmisc", bufs=2)
            nc.tensor.matmul(pb[:], lhsT=fc2_sb[:], rhs=r1[:],
                             start=True, stop=True)
            ob = wk.tile([1, cfg.n_graphs], f32, tag="ob", bufs=1)
            nc.scalar.activation(ob[:], pb[:], AF.Identity, bias=fc2b_sb[:])
            nc.sync.dma_start(t_out, ob[:])

    # Align each gather's SWDGE queue with its scheduler-assigned DMASW sem
    # lane (ucode shadow-sem accounting is per (sem, queue); a sem must only
    # ever be updated from one queue). Lane L -> queue L % 4.
    import re
    for inst in nc.inst_map.values():
        if isinstance(inst, mybir.InstDMAGatherAnt):
            si = inst.sync_info
            if si is not None and si.on_update:
                m = re.match(r"DMASW(\d+)_", si.on_update[0].ant_name)
                if m:
                    inst.queue_num = int(m.group(1)) % 4

    nc.compile()
    return nc


_PROG_CACHE = {}


def run_gat(x, edge_index, batch, weights, cfg, trace=False):
    """weights: dict from make_weights. Returns (out [n_graphs], exec_ns)."""
    from concourse.bass_utils import run_bass_kernel_spmd

    bf = ml_dtypes.bfloat16
    n = cfg.n_nodes
    x_pad = np.zeros((cfg.n_pad, IN_DIM), np.float32)
    x_pad[:n] = np.asarray(x, np.float32)
    xT = np.ascontiguousarray(x_pad.T).astype(bf)

    per_core, _ = preprocess(edge_index, batch, cfg)

    key = cfg.key()
    if key not in _PROG_CACHE:
        _PROG_CACHE[key] = build_program(cfg)
    nc = _PROG_CACHE[key]

    in_maps = []
    for c in range(cfg.n_cores):
        pc = per_core[c]
        m = dict(
            xT=xT,
            xTs=np.ascontiguousarray(xT[:, c * cfg.shard:(c + 1) * cfg.shard]),
            w1p=weights['w1p'], w2p=weights['w2p'], w3p=weights['w3p'],
            b1m=weights['b1m'], b2m=weights['b2m'], b3m=weights['b3m'],
            iota=weights['iota'], ident=weights['ident'],
            idx=pc['idx'], aldix=pc['aldix'], drel=pc['drel'], epi=pc['epi'],
            rcnt=pc['rcnt'],
            fc1=weights['fc1'], fc1b=weights['fc1b'],
            fc2=weights['fc2'], fc2b=weights['fc2b'],
        )
        for g in range(cfg.n_grp):
            m[f"grel{g}"] = pc['grel'][g]
        in_maps.append(m)

    res = run_bass_kernel_spmd(nc, in_maps, core_ids=list(range(cfg.n_cores)),
                               trace=trace)
    out = np.asarray(res.results[0]['out']).reshape(cfg.n_graphs, 1)
    run_gat.last_res = res
    return out, res.exec_time_ns


# ----------------------------------------------------------------------------
# Harness entrypoint: full (unsharded) inputs -> full output [N_GRAPHS, 1].
# Shards edges by destination across the 8 NeuronCores internally.
# ----------------------------------------------------------------------------
_DEF_CFG = None


def kernel(x, edge_index, batch,
           W1, a_src1, a_dst1, b1,
           W2, a_src2, a_dst2, b2,
           W3, a_src3, a_dst3, b3,
           fc1_w, fc1_b, fc2_w, fc2_b):
    global _DEF_CFG
    if _DEF_CFG is None:
        _DEF_CFG = Cfg()  # 50000 nodes / 800000 edges / 256 graphs / 8 cores
    cfg = _DEF_CFG
    weights = make_weights(W1, a_src1, a_dst1, b1, W2, a_src2, a_dst2, b2,
                           W3, a_src3, a_dst3, b3, fc1_w, fc1_b, fc2_w, fc2_b,
                           cfg)
    trace = bool(int(os.environ.get("GAT_BASS_TRACE", "0")))
    out, ns = run_gat(np.asarray(x), np.asarray(edge_index),
                      np.asarray(batch), weights, cfg, trace=trace)
    kernel.exec_time_ns = ns
    return out.astype(np.float32)



# revision 30
# speedup vs baseline: 1.8338x; 1.0922x over previous
"""GAT network (3 GATConv + mean-pool + MLP) as a Bass/Tile SPMD kernel on 8 TRN2 cores.

Sharding: nodes (and edges, partitioned by destination) split into 8 contiguous
node shards. Each core aggregates messages for its shard; bf16 feature tables
(with packed attention logit columns) are AllGathered between layers.

Self-contained: only needs numpy + the container's /opt/trn_rl_repo toolchain.
"""
import sys
import os

sys.path.insert(0, '/opt/trn_rl_repo')

import numpy as np
import ml_dtypes

H = 8
IN_DIM = 16


class Cfg:
    def __init__(self, n_nodes=50000, n_edges=800000, n_graphs=256,
                 n_cores=8, R=56, T=8, G=56, epi_chunk=7):
        self.n_nodes = n_nodes
        self.n_edges = n_edges
        self.n_graphs = n_graphs
        self.n_cores = n_cores
        self.shard = -(-n_nodes // (n_cores * 128)) * 128  # 128-aligned shard
        self.n_pad = self.shard * n_cores
        self.half = self.n_pad // 2
        self.C = self.shard // 128          # node tiles per shard
        self.R = R                          # ranges per pass (uniform)
        self.T = T                          # tiles (of 128 edges) per range
        self.G = G                          # tiles per gather supertile
        self.epi_chunk = epi_chunk          # node-tile cols per epilogue chunk
        assert self.C % epi_chunk == 0
        assert (R * T) % G == 0
        # layer dims
        self.F = [64, 128, 128]             # H * f_out per layer
        self.fph = [8, 16, 16]
        self.elem = [128, 256, 256]         # bf16 row width of gather tables
        self.nrhs = [72, 136, 136]          # msg cols + p cols
        self.PW = [128, 192, 192]           # evac width written to accum
        self.ACCW = 192                     # accum row width (f32, 768B)
        self.n_grp = -(-n_graphs // 128)    # graph groups for pooling

    def key(self):
        return (self.n_pad, self.n_graphs, self.n_cores, self.R, self.T,
                self.G, self.epi_chunk)


def _wrap16(a, reps=8):
    """j -> [j%16, j//16], replicated to 128 partitions."""
    w = a.reshape(-1, 16).T
    return np.ascontiguousarray(np.tile(w, (reps, 1)))


def _wrap128(a):
    """j -> [j%128, j//128]."""
    return np.ascontiguousarray(a.reshape(-1, 128).T)


def preprocess(edge_index, batch, cfg):
    """Build per-core edge-slot arrays. Returns (shared dict, per-core list)."""
    n, npad, shard, half = cfg.n_nodes, cfg.n_pad, cfg.shard, cfg.half
    R, T = cfg.R, cfg.T
    slot_e = T * 128                      # edges per range
    nslots = 2 * R * slot_e               # edge slots per core

    ei = np.asarray(edge_index)
    loops = np.arange(n, dtype=np.int64)
    src = np.concatenate([ei[0], loops])
    dst = np.concatenate([ei[1], loops])
    core = dst // shard

    per_core = []
    max_ranges = 0
    for c in range(cfg.n_cores):
        m = core == c
        srcs = src[m]
        dsts = dst[m] - c * shard
        idx_all = np.zeros(nslots, np.int16)
        aldix_all = np.zeros(2 * R * 128, np.int16)
        drel_all = np.full(nslots, -1.0, np.float32)
        epi = np.zeros(2 * shard, np.int16)
        for pi in range(2):
            pm = (srcs < half) if pi == 0 else (srcs >= half)
            s_p = srcs[pm]
            d_p = dsts[pm]
            order = np.argsort(d_p, kind='stable')
            s_p = s_p[order]
            d_p = d_p[order]
            cnt = np.bincount(d_p, minlength=shard)
            cum = np.concatenate([[0], np.cumsum(cnt)])
            bases = []
            s0 = 0
            while s0 < shard:
                e_node = int(np.searchsorted(cum, cum[s0] + slot_e,
                                             side='right')) - 1
                e_node = min(e_node, s0 + 128)
                assert e_node > s0, f"node {s0} has >{slot_e} edges"
                bases.append(s0)
                s0 = e_node
            nr = len(bases)
            max_ranges = max(max_ranges, nr)
            assert nr <= R, f"need R>={nr}"
            ends = bases[1:] + [shard]
            for r in range(R):
                b0 = bases[r] if r < nr else shard - 1
                aldix_all[(pi * R + r) * 128:(pi * R + r + 1) * 128] = \
                    np.minimum(b0 + np.arange(128), shard - 1)
                if r >= nr:
                    continue
                e0, e1 = int(cum[bases[r]]), int(cum[ends[r]])
                k = e1 - e0
                o = (pi * R + r) * slot_e
                idx_all[o:o + k] = s_p[e0:e1] - pi * half
                drel_all[o:o + k] = d_p[e0:e1] - bases[r]
            # epilogue slot-row index per node
            basearr = np.asarray(bases)
            rix = np.searchsorted(basearr, np.arange(shard), side='right') - 1
            epi[pi * shard:(pi + 1) * shard] = (
                (pi * R + rix) * 128 + (np.arange(shard) - basearr[rix])
            ).astype(np.int16)
        per_core.append(dict(
            idx=_wrap16(idx_all),
            aldix=_wrap16(aldix_all),
            drel=_wrap128(drel_all).astype(ml_dtypes.bfloat16),
            epi=_wrap16(epi),
        ))

    # batch-derived constants
    b = np.asarray(batch)
    cnt_g = np.bincount(b, minlength=cfg.n_graphs).astype(np.float32)
    rcnt_node = np.zeros(npad, np.float32)
    rcnt_node[:n] = 1.0 / np.maximum(cnt_g, 1.0)[b]
    gid = np.full(npad, -1, np.int64)
    gid[:n] = b
    for c in range(cfg.n_cores):
        sl = slice(c * shard, (c + 1) * shard)
        g_loc = gid[sl]
        grels = []
        for grp in range(cfg.n_grp):
            gr = np.where((g_loc >= grp * 128) & (g_loc < (grp + 1) * 128),
                          g_loc - grp * 128, -1).astype(np.float32)
            grels.append(_wrap128(gr).astype(ml_dtypes.bfloat16))
        per_core[c]['grel'] = grels
        per_core[c]['rcnt'] = _wrap128(rcnt_node[sl])
    return per_core, max_ranges


USE_PERM = bool(int(os.environ.get("GAT_PERM", "1")))
USE_4D = bool(int(os.environ.get("GAT_4D", "1")))


def _perm(fph):
    """[f][h] interleave: new col f*H+h <- old col h*fph+f."""
    if not USE_PERM:
        return np.arange(fph * H)
    return np.array([(c % H) * fph + c // H for c in range(fph * H)])


def make_weights(W1, a_src1, a_dst1, b1, W2, a_src2, a_dst2, b2,
                 W3, a_src3, a_dst3, b3, fc1_w, fc1_b, fc2_w, fc2_b, cfg):
    """Host-side weight packing: W' = [W@P | W@As | W@Ad] per layer.

    Msg columns are [f][h]-interleaved (head fastest) so the per-head
    broadcasts on-chip keep a packed last dim; each layer's input rows are
    permuted to match the previous layer's output order.
    """
    def pack(W, a_s, a_d):
        f = a_s.shape[1]
        As = np.zeros((H * f, H), np.float32)
        Ad = np.zeros((H * f, H), np.float32)
        for h in range(H):
            As[h * f:(h + 1) * f, h] = a_s[h]
            Ad[h * f:(h + 1) * f, h] = a_d[h]
        out = np.concatenate([W[:, _perm(f)], W @ As, W @ Ad], axis=1)
        width = 128 if out.shape[1] <= 128 else 256
        pad = np.zeros((out.shape[0], width - out.shape[1]), np.float32)
        return np.concatenate([out, pad], axis=1)

    bf = ml_dtypes.bfloat16
    p1, p2, p3 = _perm(8), _perm(16), _perm(16)
    W2r = np.asarray(W2, np.float32)[p1]     # rows follow l1 output order
    W3r = np.asarray(W3, np.float32)[p2]     # rows follow l2 output order
    fc1r = np.asarray(fc1_w, np.float32)[p3]
    sh = dict(
        w1p=pack(np.asarray(W1, np.float32), np.asarray(a_src1), np.asarray(a_dst1)).astype(bf),
        w2p=pack(W2r, np.asarray(a_src2), np.asarray(a_dst2)).astype(bf),
        w3p=pack(W3r, np.asarray(a_src3), np.asarray(a_dst3)).astype(bf),
        b1m=np.tile(np.asarray(b1, np.float32)[_perm(8)][None, :], (128, 1)),
        b2m=np.tile(np.asarray(b2, np.float32)[_perm(16)][None, :], (128, 1)),
        b3m=np.tile(np.asarray(b3, np.float32)[_perm(16)][None, :], (128, 1)),
        iota=np.tile(np.arange(128, dtype=np.float32)[None, :], (128, 1)).astype(bf),
        ident=np.eye(128, dtype=np.float32).astype(bf),
        fc1=fc1r,
        fc1b=np.asarray(fc1_b, np.float32).reshape(32, 1),
        fc2=np.asarray(fc2_w, np.float32),
        fc2b=np.asarray(fc2_b, np.float32).reshape(1, 1),
    )
    return sh


def build_program(cfg):
    import concourse.bacc as bacc
    import concourse.bass as bass
    import concourse.tile as tile
    import concourse.mybir as mybir

    f32 = mybir.dt.float32
    bf16 = mybir.dt.bfloat16
    i16 = mybir.dt.int16
    AF = mybir.ActivationFunctionType
    OP = mybir.AluOpType

    npad, shard, C, half = cfg.n_pad, cfg.shard, cfg.C, cfg.half
    R, T, G = cfg.R, cfg.T, cfg.G
    n_sup = 2 * R * T // G          # gather supertiles per layer
    sup_pass = n_sup // 2
    rpg = G // T                    # ranges per supertile
    ECH = cfg.epi_chunk
    ACCW = cfg.ACCW

    nc = bacc.Bacc("TRN2", target_bir_lowering=False, debug=False,
                   num_devices=cfg.n_cores, num_swdge_queues=4)
    # NOTE: each tile buffer's DMA-completion semaphore is locked to one SWDGE
    # queue, so queue assignment must be deterministic per buffer/region.

    def din(name, shape, dt):
        return nc.dram_tensor(name, shape, dt, kind="ExternalInput").ap()

    t_xT = din("xT", [IN_DIM, npad], bf16)
    t_xTs = din("xTs", [IN_DIM, shard], bf16)
    t_w = [din("w1p", [IN_DIM, 128], bf16),
           din("w2p", [64, 256], bf16),
           din("w3p", [128, 256], bf16)]
    t_b = [din("b1m", [128, 64], f32),
           din("b2m", [128, 128], f32),
           din("b3m", [128, 128], f32)]
    t_iota = din("iota", [128, 128], bf16)
    t_ident = din("ident", [128, 128], bf16)
    t_idx = din("idx", [128, 2 * R * T * 8], i16)
    t_aldix = din("aldix", [128, 2 * R * 8], i16)
    t_drel = din("drel", [128, 2 * R * T], bf16)
    t_epi = din("epi", [128, 2 * shard // 16], i16)
    t_grel = [din(f"grel{g}", [128, C], bf16) for g in range(cfg.n_grp)]
    t_rcnt = din("rcnt", [128, C], f32)
    t_fc1 = din("fc1", [128, 32], f32)
    t_fc1b = din("fc1b", [32, 1], f32)
    t_fc2 = din("fc2", [32, 1], f32)
    t_fc2b = din("fc2b", [1, 1], f32)
    t_out = nc.dram_tensor("out", [1, cfg.n_graphs], f32,
                           kind="ExternalOutput").ap()

    table1s = nc.dram_tensor("table1s", [npad, 128], bf16,
                             addr_space="Shared")
    tables = [None,
              nc.dram_tensor("table2", [npad, 256], bf16, addr_space="Shared"),
              nc.dram_tensor("table3", [npad, 256], bf16, addr_space="Shared")]
    aldtabs = [nc.dram_tensor(f"aldtab{i}", [shard, 128], bf16)
               for i in range(3)]
    accum = nc.dram_tensor("accum", [2 * R * 128, ACCW], f32)

    with tile.TileContext(nc) as tc:
        with tc.tile_pool(name="cst", bufs=1) as cst, \
             tc.tile_pool(name="wk", bufs=1) as wk, \
             tc.tile_pool(name="ps", bufs=1, space="PSUM") as ps, \
             tc.tile_pool(name="dram", bufs=1, space="DRAM") as dram:

            # ---- persistent constants -------------------------------------
            iota_sb = cst.tile([128, 128], bf16)
            nc.sync.dma_start(iota_sb[:], t_iota)
            iota3 = iota_sb[:].rearrange("p (o j) -> p o j", o=1)
            ident_sb = cst.tile([128, 128], bf16)
            nc.sync.dma_start(ident_sb[:], t_ident)
            w_sb = []
            for l in range(3):
                w = cst.tile([t_w[l].shape[0], t_w[l].shape[1]], bf16,
                             name=f"w{l}_sb")
                nc.sync.dma_start(w[:], t_w[l])
                w_sb.append(w)
            b_sb = []
            for l in range(3):
                bt = cst.tile([128, t_b[l].shape[1]], f32, name=f"b{l}_sb")
                nc.sync.dma_start(bt[:], t_b[l])
                b_sb.append(bt)

            grel_sb = []
            for g in range(cfg.n_grp):
                gt_ = cst.tile([128, C], bf16, name=f"grel{g}_sb")
                nc.sync.dma_start(gt_[:], t_grel[g])
                grel_sb.append(gt_)
            rcnt_sb = cst.tile([128, C], f32)
            nc.sync.dma_start(rcnt_sb[:], t_rcnt)
            fc1_sb = cst.tile([128, 32], f32)
            nc.sync.dma_start(fc1_sb[:], t_fc1)
            fc1b_sb = cst.tile([32, 1], f32)
            nc.sync.dma_start(fc1b_sb[:], t_fc1b)
            fc2_sb = cst.tile([32, 1], f32)
            nc.sync.dma_start(fc2_sb[:], t_fc2)
            fc2b_sb = cst.tile([1, 1], f32)
            nc.sync.dma_start(fc2b_sb[:], t_fc2b)

            # ---- L1: each core computes ITS shard of table1, AllGather ----
            xTs_sb = wk.tile([128, shard], bf16, tag="xt", bufs=1)
            nc.sync.dma_start(xTs_sb[0:IN_DIM, :], t_xTs)
            cin1 = dram.tile([shard, 128], bf16, tag="cin1", bufs=1)
            alds = wk.tile([128, C, 128], bf16, tag="alds", bufs=1)
            nc.vector.memset(alds[:], 0.0)
            for c in range(C):
                pm = ps.tile([128, 128], f32, tag="pmisc", bufs=2)
                nc.tensor.matmul(pm[:],
                                 lhsT=xTs_sb[0:IN_DIM, c * 128:(c + 1) * 128],
                                 rhs=w_sb[0][:], start=True, stop=True)
                tb1 = wk.tile([128, 128], bf16, tag="tb", bufs=3)
                nc.vector.tensor_copy(tb1[:], pm[:])
                nc.vector.tensor_copy(alds[:, c, 0:8], pm[:, 72:80])
                nc.sync.dma_start(cin1[c * 128:(c + 1) * 128, :], tb1[:])
            nc.sync.dma_start(
                aldtabs[0].ap().rearrange("(c p) j -> p c j", p=128), alds[:])
            nc.gpsimd.collective_compute(
                "AllGather", OP.bypass,
                replica_groups=[list(range(cfg.n_cores))],
                ins=[cin1.opt()], outs=[table1s.ap()])

            pool_sb = cst.tile([128, cfg.n_grp * 128], f32)

            # ---- layers ---------------------------------------------------
            for l in range(3):
                F, fph, elem, nrhs = cfg.F[l], cfg.fph[l], cfg.elem[l], cfg.nrhs[l]
                PW = cfg.PW[l]
                tab = tables[l]

                # EDGE PHASE
                for sp in range(n_sup):
                    pi = 0 if sp < sup_pass else 1
                    if l == 0:
                        tab_ap = table1[pi].ap()
                    else:
                        tab_ap = tab.ap()[pi * half:(pi + 1) * half, :]
                    o8 = sp * G * 8
                    rpsup = G // T
                    idxs = wk.tile([128, G * 8], i16, tag="gidx", bufs=2)
                    nc.sync.dma_start(idxs[:], t_idx[:, o8:o8 + G * 8])
                    aix = wk.tile([128, rpsup * 8], i16, tag="aix", bufs=2)
                    nc.sync.dma_start(
                        aix[:], t_aldix[:, sp * rpsup * 8:(sp + 1) * rpsup * 8])
                    drl = wk.tile([128, G], bf16, tag="drel", bufs=2)
                    nc.sync.dma_start(drl[:], t_drel[:, sp * G:(sp + 1) * G])

                    gt = wk.tile([128, G, elem], bf16, tag="gt", bufs=2)
                    PIECE = 14                    # tiles per gather: 1792 idxs
                    for g0 in range(0, G, PIECE):
                        g1 = min(g0 + PIECE, G)
                        nc.gpsimd.dma_gather(gt[:, g0:g1, :], tab_ap,
                                             idxs[:, g0 * 8:g1 * 8],
                                             num_idxs=(g1 - g0) * 128,
                                             num_idxs_reg=(g1 - g0) * 128,
                                             elem_size=elem,
                                             single_packet=False,
                                             queue_num=(g0 // PIECE) % 4)
                    aldr = wk.tile([128, rpsup, 128], bf16, tag="aldr", bufs=2)
                    nc.gpsimd.dma_gather(aldr[:], aldtabs[l].ap(), aix[:],
                                         num_idxs=rpsup * 128,
                                         num_idxs_reg=rpsup * 128,
                                         elem_size=128, single_packet=False,
                                         queue_num=sp % 2)

                    # expand al_d per edge: per tile, psum_ald = S_T @ aldr
                    aldx = wk.tile([128, G, 8], f32, tag="aldx", bufs=2)
                    Ss = []
                    for rr in range(rpsup):
                        pald = ps.tile([128, T * 8], f32, tag="pmisc", bufs=2)
                        Se = wk.tile([128, T, 128], bf16, tag="S",
                                     bufs=rpsup)
                        d3 = drl[:, rr * T:(rr + 1) * T].rearrange(
                            "p (t o) -> p t o", o=1)
                        nc.vector.tensor_tensor(
                            out=Se[:], in0=d3.to_broadcast([128, T, 128]),
                            in1=iota3.to_broadcast([128, T, 128]),
                            op=OP.is_equal)
                        Ss.append(Se)
                        pst8 = ps.tile([128, T, 128], bf16, tag="stpool",
                                       bufs=2)
                        for t in range(T):
                            nc.tensor.transpose(pst8[:, t, :], Se[:, t, :],
                                                ident_sb[:])
                        st8 = wk.tile([128, T, 128], bf16, tag="st", bufs=3)
                        nc.scalar.activation(st8[:], pst8[:], AF.Copy)
                        for t in range(T):
                            nc.tensor.matmul(pald[:, t * 8:(t + 1) * 8],
                                             lhsT=st8[:, t, :],
                                             rhs=aldr[:, rr, 0:8],
                                             start=True, stop=True)
                        nc.vector.tensor_copy(aldx[:, rr * T:(rr + 1) * T, :],
                                              pald[:])

                    # p = exp(lrelu(als+ald)) = max(exp(x), exp(0.2x))
                    lg = wk.tile([128, G, 8], f32, tag="lg", bufs=3)
                    nc.vector.tensor_tensor(out=lg[:], in0=gt[:, :, F:F + 8],
                                            in1=aldx[:], op=OP.add)
                    e1 = wk.tile([128, G, 8], f32, tag="lg", bufs=3)
                    nc.scalar.activation(e1[:], lg[:], AF.Exp)
                    e2 = wk.tile([128, G, 8], f32, tag="lg", bufs=3)
                    nc.scalar.activation(e2[:], lg[:], AF.Exp, scale=0.2)
                    nc.vector.tensor_tensor(out=gt[:, :, F:F + 8], in0=e1[:],
                                            in1=e2[:], op=OP.max)
                    # weight messages by p per head ([f][h]-interleaved cols:
                    # head index is the fastest axis, so the broadcast keeps a
                    # packed last dim and the DVE runs in 2x mode)
                    if USE_PERM and USE_4D:
                        gt4 = gt[:, :, 0:F].rearrange("p g (f h) -> p g f h",
                                                      h=H)
                        p4 = gt[:, :, F:F + 8].rearrange(
                            "p g (o h) -> p g o h", o=1)
                        nc.vector.tensor_tensor(
                            out=gt4, in0=gt4,
                            in1=p4.to_broadcast([128, G, fph, H]),
                            op=OP.mult)
                    elif USE_PERM:
                        for f in range(fph):
                            nc.vector.tensor_tensor(
                                out=gt[:, :, f * H:(f + 1) * H],
                                in0=gt[:, :, f * H:(f + 1) * H],
                                in1=gt[:, :, F:F + 8],
                                op=OP.mult)
                    else:
                        for h in range(H):
                            nc.vector.tensor_tensor(
                                out=gt[:, :, h * fph:(h + 1) * fph],
                                in0=gt[:, :, h * fph:(h + 1) * fph],
                                in1=gt[:, :, F + h:F + h + 1].to_broadcast(
                                    [128, G, fph]),
                                op=OP.mult)

                    ev = wk.tile([128, rpg, nrhs], f32, tag="ev", bufs=2)
                    for rr in range(rpg):
                        S = Ss[rr]
                        pacc = ps.tile([128, nrhs], f32, tag="pacc", bufs=2)
                        for t in range(T):
                            nc.tensor.matmul(pacc[:], lhsT=S[:, t, :],
                                             rhs=gt[:, rr * T + t, 0:nrhs],
                                             start=(t == 0), stop=(t == T - 1))
                        nc.vector.tensor_copy(ev[:, rr, :], pacc[:])
                    nc.sync.dma_start(
                        accum.ap()[sp * rpg * 128:(sp + 1) * rpg * 128, 0:nrhs]
                        .rearrange("(s p) w -> p s w", p=128),
                        ev[:])

                # EPILOGUE (table phase / pooling interleaved per chunk)
                if l < 2:
                    xT_sb = wk.tile([128, shard], bf16, tag="xt", bufs=1)
                    cin = dram.tile([shard, 256], bf16, tag="cin", bufs=1)
                    alds2 = wk.tile([128, C, 128], bf16, tag="alds", bufs=1)
                    nc.vector.memset(alds2[:], 0.0)
                else:
                    pp_ps = [ps.tile([128, 128], f32, tag="poolp", bufs=2,
                                     name=f"poolp{_g}")
                             for _g in range(cfg.n_grp)]
                h_bfs = []
                for ch in range(C // ECH):
                    oc = ch * ECH * 8
                    e_lo = wk.tile([128, ECH * 8], i16, tag="ei", bufs=2)
                    nc.sync.dma_start(e_lo[:], t_epi[:, oc:oc + ECH * 8])
                    e_hi = wk.tile([128, ECH * 8], i16, tag="ei2", bufs=2)
                    nc.sync.dma_start(
                        e_hi[:],
                        t_epi[:, shard // 16 + oc:shard // 16 + oc + ECH * 8])
                    glo = wk.tile([128, ECH, PW], f32, tag="eg", bufs=2)
                    nc.gpsimd.dma_gather(glo[:], accum.ap()[:, 0:PW], e_lo[:],
                                         num_idxs=ECH * 128,
                                         num_idxs_reg=ECH * 128,
                                         elem_size=PW, elem_step=ACCW,
                                         single_packet=False, queue_num=2)
                    ghi = wk.tile([128, ECH, PW], f32, tag="eg", bufs=2)
                    nc.gpsimd.dma_gather(ghi[:], accum.ap()[:, 0:PW], e_hi[:],
                                         num_idxs=ECH * 128,
                                         num_idxs_reg=ECH * 128,
                                         elem_size=PW, elem_step=ACCW,
                                         single_packet=False, queue_num=3)
                    acc = wk.tile([128, ECH, nrhs], f32, tag="eacc", bufs=2)
                    nc.vector.tensor_tensor(out=acc[:], in0=glo[:, :, 0:nrhs],
                                            in1=ghi[:, :, 0:nrhs], op=OP.add)
                    rec = wk.tile([128, ECH, 8], f32, tag="rec", bufs=2)
                    nc.vector.tensor_scalar_add(rec[:], acc[:, :, F:F + 8],
                                                1e-30)
                    nc.vector.reciprocal(rec[:], rec[:])
                    if USE_PERM and USE_4D:
                        acc4 = acc[:, :, 0:F].rearrange(
                            "p e (f h) -> p e f h", h=H)
                        rec4 = rec[:].rearrange("p e (o h) -> p e o h", o=1)
                        nc.vector.tensor_tensor(
                            out=acc4, in0=acc4,
                            in1=rec4.to_broadcast([128, ECH, fph, H]),
                            op=OP.mult)
                    elif USE_PERM:
                        for f in range(fph):
                            nc.vector.tensor_tensor(
                                out=acc[:, :, f * H:(f + 1) * H],
                                in0=acc[:, :, f * H:(f + 1) * H],
                                in1=rec[:],
                                op=OP.mult)
                    else:
                        for h in range(H):
                            nc.vector.tensor_tensor(
                                out=acc[:, :, h * fph:(h + 1) * fph],
                                in0=acc[:, :, h * fph:(h + 1) * fph],
                                in1=rec[:, :, h:h + 1].to_broadcast(
                                    [128, ECH, fph]),
                                op=OP.mult)
                    nc.vector.tensor_tensor(
                        out=acc[:, :, 0:F], in0=acc[:, :, 0:F],
                        in1=b_sb[l][:].rearrange("p (o j) -> p o j", o=1)
                        .to_broadcast([128, ECH, F]),
                        op=OP.add)
                    # ELU: exp(min(x,0)) + max(x,0) - 1
                    t1 = wk.tile([128, ECH, F], f32, tag="et1", bufs=3)
                    nc.vector.tensor_scalar_min(t1[:], acc[:, :, 0:F], 0.0)
                    t2 = wk.tile([128, ECH, F], f32, tag="et1", bufs=3)
                    nc.scalar.activation(t2[:], t1[:], AF.Exp)
                    nc.vector.tensor_scalar_max(acc[:, :, 0:F],
                                                acc[:, :, 0:F], 0.0)
                    nc.vector.tensor_tensor(out=t2[:], in0=t2[:],
                                            in1=acc[:, :, 0:F], op=OP.add)
                    nc.vector.tensor_scalar_add(t2[:], t2[:], -1.0)
                    if l == 2:
                        nc.vector.tensor_tensor(
                            out=t2[:], in0=t2[:],
                            in1=rcnt_sb[:, ch * ECH:(ch + 1) * ECH]
                            .rearrange("p (t o) -> p t o", o=1)
                            .to_broadcast([128, ECH, F]),
                            op=OP.mult)
                    h_bf = wk.tile([128, ECH, F], bf16, tag="hbf",
                                   bufs=C // ECH)
                    nc.vector.tensor_copy(h_bf[:], t2[:])
                    h_bfs.append(h_bf)

                    if l < 2:
                        # table chunk: transpose + W' matmul + cin write
                        for cc in range(ECH):
                            c = ch * ECH + cc
                            pt = ps.tile([128, 128], bf16, tag="stpool",
                                         bufs=2)
                            nc.tensor.transpose(pt[0:F, :], h_bf[:, cc, :],
                                                ident_sb[:])
                            nc.vector.tensor_copy(
                                xT_sb[0:F, c * 128:(c + 1) * 128], pt[0:F, :])
                            pm = ps.tile([128, 256], f32, tag="pmisc", bufs=2)
                            nc.tensor.matmul(
                                pm[:], lhsT=xT_sb[0:F, c * 128:(c + 1) * 128],
                                rhs=w_sb[l + 1][:], start=True, stop=True)
                            tb = wk.tile([128, 256], bf16, tag="tb", bufs=3)
                            nc.vector.tensor_copy(tb[:], pm[:])
                            nc.vector.tensor_copy(alds2[:, c, 0:8],
                                                  pm[:, 136:144])
                            nc.sync.dma_start(cin[c * 128:(c + 1) * 128, :],
                                              tb[:])
                    else:
                        # pooling chunk: accumulate graph sums in psum
                        n_ch = C // ECH
                        for grp in range(cfg.n_grp):
                            spc = wk.tile([128, ECH, 128], bf16, tag="spc",
                                          bufs=2)
                            g3 = grel_sb[grp][:, ch * ECH:(ch + 1) * ECH] \
                                .rearrange("p (t o) -> p t o", o=1)
                            nc.vector.tensor_tensor(
                                out=spc[:],
                                in0=g3.to_broadcast([128, ECH, 128]),
                                in1=iota3.to_broadcast([128, ECH, 128]),
                                op=OP.is_equal)
                            for cc in range(ECH):
                                nc.tensor.matmul(
                                    pp_ps[grp][:], lhsT=h_bf[:, cc, :],
                                    rhs=spc[:, cc, :],
                                    start=(ch == 0 and cc == 0),
                                    stop=(ch == n_ch - 1 and cc == ECH - 1))

                if l < 2:
                    nc.sync.dma_start(
                        aldtabs[l + 1].ap()
                        .rearrange("(c p) j -> p c j", p=128), alds2[:])
                    nc.gpsimd.collective_compute(
                        "AllGather", OP.bypass,
                        replica_groups=[list(range(cfg.n_cores))],
                        ins=[cin.opt()], outs=[tables[l + 1].ap()])
                else:
                    for grp in range(cfg.n_grp):
                        nc.vector.tensor_copy(
                            pool_sb[:, grp * 128:(grp + 1) * 128],
                            pp_ps[grp][:])

            # AllReduce pooled sums, then the MLP on every core
            cin2 = dram.tile([128, cfg.n_grp * 128], f32, tag="cin2", bufs=1)
            cred = dram.tile([128, cfg.n_grp * 128], f32, tag="cred", bufs=1)
            nc.sync.dma_start(cin2[:], pool_sb[:])
            nc.gpsimd.collective_compute(
                "AllReduce", OP.add,
                replica_groups=[list(range(cfg.n_cores))],
                ins=[cin2.opt()], outs=[cred.opt()])
            pool2 = wk.tile([128, cfg.n_grp * 128], f32, tag="pool2", bufs=1)
            nc.sync.dma_start(pool2[:], cred[:])
            pa = ps.tile([32, cfg.n_graphs], f32, tag="pmisc", bufs=2)
            nc.tensor.matmul(pa[:], lhsT=fc1_sb[:], rhs=pool2[:, 0:cfg.n_graphs],
                             start=True, stop=True)
            r1 = wk.tile([32, cfg.n_graphs], f32, tag="r1", bufs=1)
            nc.scalar.activation(r1[:], pa[:], AF.Relu, bias=fc1b_sb[:])
            pb = ps.tile([1, cfg.n_graphs], f32, tag="pmisc", bufs=2)
            nc.tensor.matmul(pb[:], lhsT=fc2_sb[:], rhs=r1[:],
                             start=True, stop=True)
            ob = wk.tile([1, cfg.n_graphs], f32, tag="ob", bufs=1)
            nc.scalar.activation(ob[:], pb[:], AF.Identity, bias=fc2b_sb[:])
            nc.sync.dma_start(t_out, ob[:])

    # Align each gather's SWDGE queue with its scheduler-assigned DMASW sem
    # lane (ucode shadow-sem accounting is per (sem, queue); a sem must only
    # ever be updated from one queue). Lane L -> queue L % 4.
    import re
    for inst in nc.inst_map.values():
        if isinstance(inst, mybir.InstDMAGatherAnt):
            si = inst.sync_info
            if si is not None and si.on_update:
                m = re.match(r"DMASW(\d+)_", si.on_update[0].ant_name)
                if m:
                    inst.queue_num = int(m.group(1)) % 4

    nc.compile()
    return nc


_PROG_CACHE = {}


def run_gat(x, edge_index, batch, weights, cfg, trace=False):
    """weights: dict from make_weights. Returns (out [n_graphs], exec_ns)."""
    from concourse.bass_utils import run_bass_kernel_spmd

    bf = ml_dtypes.bfloat16
    n = cfg.n_nodes
    x_pad = np.zeros((cfg.n_pad, IN_DIM), np.float32)
    x_pad[:n] = np.asarray(x, np.float32)
    xT = np.ascontiguousarray(x_pad.T).astype(bf)

    per_core, _ = preprocess(edge_index, batch, cfg)

    key = cfg.key()
    if key not in _PROG_CACHE:
        _PROG_CACHE[key] = build_program(cfg)
    nc = _PROG_CACHE[key]

    in_maps = []
    for c in range(cfg.n_cores):
        pc = per_core[c]
        m = dict(
            xT=xT,
            xTs=np.ascontiguousarray(xT[:, c * cfg.shard:(c + 1) * cfg.shard]),
            w1p=weights['w1p'], w2p=weights['w2p'], w3p=weights['w3p'],
            b1m=weights['b1m'], b2m=weights['b2m'], b3m=weights['b3m'],
            iota=weights['iota'], ident=weights['ident'],
            idx=pc['idx'], aldix=pc['aldix'], drel=pc['drel'], epi=pc['epi'],
            rcnt=pc['rcnt'],
            fc1=weights['fc1'], fc1b=weights['fc1b'],
            fc2=weights['fc2'], fc2b=weights['fc2b'],
        )
        for g in range(cfg.n_grp):
            m[f"grel{g}"] = pc['grel'][g]
        in_maps.append(m)

    res = run_bass_kernel_spmd(nc, in_maps, core_ids=list(range(cfg.n_cores)),
                               trace=trace)
    out = np.asarray(res.results[0]['out']).reshape(cfg.n_graphs, 1)
    run_gat.last_res = res
    return out, res.exec_time_ns


# ----------------------------------------------------------------------------
# Harness entrypoint: full (unsharded) inputs -> full output [N_GRAPHS, 1].
# Shards edges by destination across the 8 NeuronCores internally.
# ----------------------------------------------------------------------------
_DEF_CFG = None


def kernel(x, edge_index, batch,
           W1, a_src1, a_dst1, b1,
           W2, a_src2, a_dst2, b2,
           W3, a_src3, a_dst3, b3,
           fc1_w, fc1_b, fc2_w, fc2_b):
    global _DEF_CFG
    if _DEF_CFG is None:
        _DEF_CFG = Cfg()  # 50000 nodes / 800000 edges / 256 graphs / 8 cores
    cfg = _DEF_CFG
    weights = make_weights(W1, a_src1, a_dst1, b1, W2, a_src2, a_dst2, b2,
                           W3, a_src3, a_dst3, b3, fc1_w, fc1_b, fc2_w, fc2_b,
                           cfg)
    trace = bool(int(os.environ.get("GAT_BASS_TRACE", "0")))
    out, ns = run_gat(np.asarray(x), np.asarray(edge_index),
                      np.asarray(batch), weights, cfg, trace=trace)
    kernel.exec_time_ns = ns
    return out.astype(np.float32)

